# revision 1
# baseline (speedup 1.0000x reference)
"""DAG-GNN level-sweep kernel for Trainium2 (8 NeuronCores, Bass/Tile).

Structure:
  - Host (numpy): build level plans — per-level gather index arrays, window/chunk
    structure (uniform across cores for SPMD), initial-value code counts, readout
    layout.
  - Device: per level: indirect-DMA gather of source rows from table T ->
    PE matmul vs on-chip one-hot selection matrices (segment-sum + transpose in
    one op) -> PE matmul vs extended weights (aggregate @ W + counts @ (h0@W)
    + indeg*b) -> DMA -> 8-core AllGather into T's level block.
  - Readout: gather final rows of output nodes, per-graph sum via PE matmul
    against graph-indicator, per-graph max via DVE max-tree + PE transpose +
    free-axis reduce, tiny AllGather of per-core partials, combine.
"""

import sys

if "/opt/trn_rl_repo" not in sys.path:
    sys.path.insert(0, "/opt/trn_rl_repo")

import math

import numpy as np

# structural constants of the nn.Module (match reference)
B = 16   # graphs per batch
LF = 12  # forward topological levels
LB = 12  # backward topological levels
D = 64   # hidden dim
C = 8    # NeuronCores
P = 128  # SBUF partitions

# T table header rows
_ZROW = 0      # all-zero row (gather padding)
_H0 = 1        # 9 rows of initial-encoding table, indexed by 3*nt + nip
_MIROW = 11    # -float32.max row (readout max padding)
_HDR = 12


def _ceil(a, b):
    return -(-a // b)


_W = 32767  # dma_gather int16 source-window size (rows)


class _Lvl:
    """Static per-level plan (identical across cores)."""

    __slots__ = (
        "sweep", "l", "Lp", "tb", "ag0", "cnt0", "ww", "nwin", "nch", "ch0",
        "win_srcs", "extras", "segs", "acol0", "ccol0", "stg0", "A_pad",
    )


def _two_hop(gpos, idxA_list, idxC_list, seg_meta):
    """Build phase-A (per source window) + phase-C (permutation) index arrays.

    gpos: [C, S] int64 final-order T positions (-1 = padding slot).
    Appends int16 wrapped index blocks to idxA_list / idxC_list and returns
    (segs, A_pad): segs = [(window_base_row, n_slots)], A_pad = staging rows.
    Uniform across cores; per-core shortfalls padded with index 0.
    """
    Csz, S = gpos.shape
    sw = np.where(gpos >= 0, gpos // _W, -1)
    sws = sorted(set(sw[sw >= 0].tolist()))
    segs = []
    posA = np.zeros((Csz, S), np.int64)  # final slot -> phase-A position
    base = 0
    for s in sws:
        cnts = [(sw[c] == s).sum() for c in range(Csz)]
        n = _ceil(max(cnts), 128) * 128 if max(cnts) else 0
        if n == 0:
            continue
        for c in range(Csz):
            m = sw[c] == s
            posA[c, m] = base + np.arange(int(cnts[c]))
        segs.append((s * _W, n))
        base += n
    A_pad = base
    # idxA: per phase-A slot, offset within its window (0 for padding)
    idxA = np.zeros((Csz, max(A_pad, 128)), np.int16)
    for c in range(Csz):
        m = sw[c] >= 0
        idxA[c, posA[c, m]] = (gpos[c, m] % _W).astype(np.int16)
    A_pad = max(A_pad, 128)
    if not segs:
        segs = [(0, 128)]
    # idxC: final slot -> phase-A position (0 for padding)
    idxC = np.zeros((Csz, S), np.int16)
    for c in range(Csz):
        m = sw[c] >= 0
        idxC[c, m] = posA[c, m].astype(np.int16)
    # wrap into [128, n/16] layout, replicated into all 8 channel groups
    def wrap(a):
        n = a.shape[1]
        w = a.reshape(Csz, n // 16, 16).transpose(0, 2, 1)  # [C, 16, n/16]
        return np.ascontiguousarray(np.tile(w, (1, 8, 1)))  # [C, 128, n/16]
    idxA_list.append(wrap(idxA))
    idxC_list.append(wrap(idxC))
    seg_meta.append((segs, A_pad))
    return segs, A_pad


def _preprocess(node_type, num_inverted_predecessors, edge_index,
                forward_level, backward_level, batch,
                W_enc, b_enc, W_f, b_f, W_b, b_b):
    N = int(node_type.shape[0])
    nt = np.asarray(node_type).astype(np.int64)
    nip = np.asarray(num_inverted_predecessors).astype(np.int64)
    fl = np.asarray(forward_level).astype(np.int64)
    bl = np.asarray(backward_level).astype(np.int64)
    bt = np.asarray(batch).astype(np.int64)
    src = np.asarray(edge_index[0]).astype(np.int64)
    dst = np.asarray(edge_index[1]).astype(np.int64)
    code = nt * 3 + nip  # in [0, 9)

    # ---------------- node positions in T ----------------
    posf = np.full(N, -1, np.int64)
    posb = np.full(N, -1, np.int64)
    coref = np.zeros(N, np.int64)
    rankf = np.zeros(N, np.int64)
    coreb = np.zeros(N, np.int64)
    rankb = np.zeros(N, np.int64)

    tbase = _HDR
    lvl_meta = []  # (sweep, l, Lp, tbase)
    for sweep, lv, pos, core, rank, nl in (
        ("f", fl, posf, coref, rankf, LF),
        ("b", bl, posb, coreb, rankb, LB),
    ):
        for l in range(1, nl):
            idx = np.flatnonzero(lv == l)
            n_l = idx.size
            if n_l == 0:
                lvl_meta.append((sweep, l, 0, tbase))
                continue
            Lp = _ceil(_ceil(n_l, C), P) * P
            c = np.arange(n_l) % C
            r = np.arange(n_l) // C
            pos[idx] = tbase + c * Lp + r
            core[idx] = c
            rank[idx] = r
            lvl_meta.append((sweep, l, Lp, tbase))
            tbase += C * Lp
    T_ROWS = tbase

    # ---------------- per-level edge structures ----------------
    plans = []
    cdst_blocks = []   # [C, P, nch] float32
    cnts_blocks = []   # [C, 10, Lp] float32
    idxA_list = []     # [C, 128, n/16] int16 per level
    idxC_list = []
    seg_meta = []
    acol0 = ccol0 = stg0 = 0
    cnt0 = 0
    ch0 = 0
    fl_dst = fl[dst]
    bl_src = bl[src]
    for (sweep, l, Lp, tb) in lvl_meta:
        if Lp == 0:
            continue
        pl = _Lvl()
        pl.sweep, pl.l, pl.Lp, pl.tb = sweep, l, Lp, tb
        pl.ag0 = cnt0   # agin rows share the Lp-prefix-sum layout
        pl.cnt0 = cnt0
        if sweep == "f":
            em = fl_dst == l
            un = dst[em]
            dn = src[em]
            ucore = coref[un]
            urank = rankf[un]
            gat = (fl[dn] >= 1) & (fl[dn] < l)
            gpos_all = posf[dn]
        else:
            em = bl_src == l
            un = src[em]
            dn = dst[em]
            ucore = coreb[un]
            urank = rankb[un]
            upd_b = (bl[dn] >= 1) & (bl[dn] < l)
            upd_f = (~upd_b) & (fl[dn] >= 1)
            gat = upd_b | upd_f
            gpos_all = np.where(upd_b, posb[dn], posf[dn])

        # counts: initial-valued sources by code, plus total indegree (bias)
        cnt = np.zeros((C, Lp, 10), np.float32)
        i0 = ~gat
        np.add.at(cnt, (ucore[i0], urank[i0], code[dn[i0]]), 1.0)
        np.add.at(cnt, (ucore, urank, 9), 1.0)
        cnts_blocks.append(np.ascontiguousarray(cnt.transpose(0, 2, 1)))
        cnt0 += Lp

        # gather slots, grouped per destination-rank window, padded uniform
        gc = ucore[gat]
        gr = urank[gat]
        gp = gpos_all[gat]
        if gp.size == 0:
            pl.ww, pl.nwin, pl.nch, pl.ch0 = 128, 0, 0, ch0
            pl.win_srcs, pl.extras = [], []
            pl.segs, pl.acol0, pl.ccol0, pl.stg0, pl.A_pad = [], 0, 0, 0, 0
            plans.append(pl)
            continue
        nslot_c = np.bincount(gc, minlength=C)
        dens = nslot_c.max() / Lp
        ww = min(4, max(1, _ceil(96, int(128 * dens) + 1))) * 128
        nwin = _ceil(Lp, ww)
        w_e = gr // ww
        cw = np.zeros((C, nwin), np.int64)
        np.add.at(cw, (gc, w_e), 1)
        Pw = cw.max(axis=0)
        S_total = int(Pw.sum())
        nch = _ceil(S_total, P)
        S_pad = nch * P
        w_slot = np.repeat(np.arange(nwin), Pw)
        w_slot = np.concatenate(
            [w_slot, np.full(S_pad - S_total, max(0, nwin - 1), np.int64)])
        win_base = np.concatenate([[0], np.cumsum(Pw)])[:-1]

        r_sl = np.full((C, S_pad), -1, np.int64)
        g_sl = np.zeros((C, S_pad), np.int64)
        order = np.lexsort((gr, w_e, gc))
        gc_o, gr_o, gp_o, w_o = gc[order], gr[order], gp[order], w_e[order]
        grp = gc_o * nwin + w_o
        counts = np.bincount(grp, minlength=C * nwin)
        gstart = np.concatenate([[0], np.cumsum(counts)])[:-1]
        seq = np.arange(gc_o.size) - gstart[grp]
        slotpos = win_base[w_o] + seq
        r_sl[gc_o, slotpos] = gr_o
        g_sl[gc_o, slotpos] = gp_o

        ch_wfirst = w_slot[::P]  # [nch]
        slot_chunk = np.arange(S_pad) // P
        cd = r_sl.astype(np.float64) - (ww * ch_wfirst[slot_chunk])[None, :]
        cd[r_sl < 0] = -1.0

        cdst_blocks.append(np.ascontiguousarray(
            cd.reshape(C, nch, P).transpose(0, 2, 1)).astype(np.float32))
        gpos_lin = np.where(r_sl >= 0, g_sl, -1)
        segs, A_pad = _two_hop(gpos_lin, idxA_list, idxC_list, seg_meta)
        pl.segs, pl.A_pad = segs, A_pad
        pl.acol0, pl.ccol0, pl.stg0 = acol0, ccol0, stg0
        acol0 += idxA_list[-1].shape[2]
        ccol0 += idxC_list[-1].shape[2]
        stg0 += A_pad

        win_srcs = [[] for _ in range(nwin)]
        extras = []
        for j in range(nch):
            ws = np.unique(w_slot[j * P:(j + 1) * P])
            wf = int(ch_wfirst[j])
            for w in ws:
                k = int(w) - wf
                win_srcs[int(w)].append((j, k))
                if k >= 1:
                    extras.append((j, k))
        pl.ww, pl.nwin, pl.nch, pl.ch0 = ww, nwin, nch, ch0
        pl.win_srcs, pl.extras = win_srcs, extras
        ch0 += nch
        plans.append(pl)

    TOTCH = max(1, ch0)
    CNT_TOT = cnt0
    cdst_all = (np.concatenate(cdst_blocks, axis=2) if cdst_blocks
                else np.full((C, P, 1), -1.0, np.float32))
    if cdst_all.shape[2] < TOTCH:  # pad to TOTCH (degenerate case)
        pad = TOTCH - cdst_all.shape[2]
        cdst_all = np.concatenate(
            [cdst_all, np.full((C, P, pad), -1.0, np.float32)], axis=2)
    cnts_all = np.concatenate(cnts_blocks, axis=2)

    # ---------------- readout layout ----------------
    onodes = np.flatnonzero(nt == 1)
    og = bt[onodes]
    fpos = np.where(bl[onodes] >= 1, posb[onodes],
                    np.where(fl[onodes] >= 1, posf[onodes],
                             _H0 + code[onodes]))
    graph_chunks = []
    kg_list = []
    for g in range(B):
        n_g = int((og == g).sum())
        kg = max(1, _ceil(_ceil(max(n_g, 1), C), P))
        kg_list.append(kg)
    NRCH = int(np.sum(kg_list))
    c0s = np.concatenate([[0], np.cumsum(kg_list)])[:-1]
    roff = np.full((C, P, NRCH), -1, np.int64)
    rgid = np.full((C, P, NRCH), -1.0, np.float32)
    for g in range(B):
        m = og == g
        npos = fpos[m]
        n_g = npos.size
        graph_chunks.append((int(c0s[g]), kg_list[g]))
        if n_g == 0:
            continue
        c = np.arange(n_g) % C
        sq = np.arange(n_g) // C
        j = sq // P
        p = sq % P
        roff[c, p, int(c0s[g]) + j] = npos
        rgid[c, p, int(c0s[g]) + j] = g
    # padding readout slots gather the -inf row (neutral for max; rgid=-1
    # keeps them out of the sum)
    roff[roff < 0] = _MIROW
    # readout two-hop (linear slot index s = j*128 + p)
    roff_lin = np.ascontiguousarray(
        roff.transpose(0, 2, 1)).reshape(C, NRCH * P)
    ro_segs, ro_A = _two_hop(roff_lin, idxA_list, idxC_list, seg_meta)
    ro_acol0, ro_ccol0, ro_stg0 = acol0, ccol0, stg0
    acol0 += idxA_list[-1].shape[2]
    ccol0 += idxC_list[-1].shape[2]
    stg0 += ro_A

    # ---------------- weight-derived constants ----------------
    W_enc = np.asarray(W_enc, np.float32)
    b_enc = np.asarray(b_enc, np.float32)
    W_f = np.asarray(W_f, np.float32)
    b_f = np.asarray(b_f, np.float32)
    W_b = np.asarray(W_b, np.float32)
    b_b = np.asarray(b_b, np.float32)
    h0_tab = np.zeros((9, D), np.float32)
    for cc in range(9):
        h0_tab[cc] = (cc // 3) * W_enc[0] + (cc % 3) * W_enc[1] + b_enc
    tab = np.zeros((_HDR, D), np.float32)
    tab[_H0:_H0 + 9] = h0_tab
    tab[_MIROW] = np.finfo(np.float32).min
    wf_ext = np.concatenate([W_f, h0_tab @ W_f, b_f[None, :]], axis=0)
    wb_ext = np.concatenate([W_b, h0_tab @ W_b, b_b[None, :]], axis=0)
    wext = np.ascontiguousarray(np.concatenate([wf_ext, wb_ext], axis=1))

    iota512 = np.ascontiguousarray(
        np.tile(np.arange(512, dtype=np.float32), (P, 1)))
    iota16 = np.ascontiguousarray(
        np.tile(np.arange(16, dtype=np.float32), (P, 1)))

    idxA_all = np.concatenate(idxA_list, axis=2)
    idxC_all = np.concatenate(idxC_list, axis=2)
    meta = dict(
        plans=plans, graph_chunks=graph_chunks,
        T_ROWS=T_ROWS, AG_ROWS=max(1, CNT_TOT), TOTCH=TOTCH,
        CNT_TOT=max(1, CNT_TOT), NRCH=NRCH,
        AW=idxA_all.shape[2], CW=idxC_all.shape[2], STG=stg0,
        ro=dict(segs=ro_segs, A_pad=ro_A, acol0=ro_acol0,
                ccol0=ro_ccol0, stg0=ro_stg0),
    )
    arrays = dict(
        tab=tab, wext=wext, iota512=iota512, iota16=iota16,
        idxA=idxA_all, idxC=idxC_all, cdst=cdst_all, cnts=cnts_all,
        rgid=rgid,
    )
    return meta, arrays


# ---------------------------------------------------------------------------
# pure-numpy execution of the plan (host self-check / debugging)
# ---------------------------------------------------------------------------

def _gather_two_hop(T, arrays, c, segs, A_pad, acol0, ccol0, n_slots):
    """numpy reference of the device two-hop gather; returns [n_slots, D]."""
    idxA = arrays["idxA"][c][0:16, :]
    idxC = arrays["idxC"][c][0:16, :]
    stg = np.zeros((A_pad, D), T.dtype)
    base = 0
    for (swb, n) in segs:
        cols = slice(acol0 + base // 16, acol0 + (base + n) // 16)
        off = idxA[:, cols].T.reshape(-1)[:n].astype(np.int64)
        stg[base:base + n] = T[swb + off]
        base += n
    perm = idxC[:, ccol0:ccol0 + n_slots // 16].T.reshape(-1).astype(np.int64)
    return stg[perm]


def _simulate_plan(meta, arrays, return_T=False):
    T = np.zeros((meta["T_ROWS"], D), np.float32)
    T[0:_HDR] = arrays["tab"]
    wext = arrays["wext"]
    for pl in meta["plans"]:
        wmat = wext[:, 0:D] if pl.sweep == "f" else wext[:, D:2 * D]
        blocks = []
        for c in range(C):
            lhs = np.zeros((74, pl.Lp), np.float32)
            lhs[64:74] = arrays["cnts"][c, :, pl.cnt0:pl.cnt0 + pl.Lp]
            if pl.nch > 0:
                cdv = arrays["cdst"][c][:, pl.ch0:pl.ch0 + pl.nch]
                G_lin = _gather_two_hop(T, arrays, c, pl.segs, pl.A_pad,
                                        pl.acol0, pl.ccol0, pl.nch * P)
                G = G_lin.reshape(pl.nch, P, D).transpose(1, 0, 2)
                written = np.zeros(pl.nwin, bool)
                aggT = np.zeros((64, pl.Lp), np.float32)
                for w in range(pl.nwin):
                    width = min(pl.ww, pl.Lp - w * pl.ww)
                    for (j, k) in pl.win_srcs[w]:
                        S = (cdv[:, j:j + 1] ==
                             (np.arange(width) + k * pl.ww)[None, :])
                        aggT[:, w * pl.ww:w * pl.ww + width] += (
                            G[:, j, :].T @ S.astype(np.float32))
                        written[w] = True
                for w in range(pl.nwin):
                    if not written[w]:
                        width = min(pl.ww, pl.Lp - w * pl.ww)
                        aggT[:, w * pl.ww:w * pl.ww + width] = 0.0
                lhs[0:64] = aggT
            blocks.append(lhs.T @ wmat)  # [Lp, D]
        T[pl.tb:pl.tb + C * pl.Lp] = np.concatenate(blocks, axis=0)
    # readout
    maxp = np.full((B, D), np.finfo(np.float32).min, np.float32)
    sump = np.zeros((B, D), np.float32)
    ro = meta["ro"]
    for c in range(C):
        R_lin = _gather_two_hop(T, arrays, c, ro["segs"], ro["A_pad"],
                                ro["acol0"], ro["ccol0"],
                                meta["NRCH"] * P)
        R = R_lin.reshape(meta["NRCH"], P, D).transpose(1, 0, 2)
        gid = arrays["rgid"][c]           # [P, NRCH]
        for g, (c0, kg) in enumerate(meta["graph_chunks"]):
            sl = R[:, c0:c0 + kg, :]
            maxp[g] = np.maximum(maxp[g], sl.max(axis=(0, 1)))
            msk = (gid[:, c0:c0 + kg] == g).astype(np.float32)
            sump[g] += np.einsum("pk,pkd->d", msk, sl)
    out = np.concatenate([maxp, sump], axis=1)
    return (out, T) if return_T else out


# ---------------------------------------------------------------------------
# Bass program
# ---------------------------------------------------------------------------

def _build(meta):
    import concourse.bass as bass
    import concourse.mybir as mybir
    from concourse import bacc, tile
    from concourse.masks import make_identity

    f32 = mybir.dt.float32
    i16 = mybir.dt.int16
    AX = mybir.AxisListType
    OP = mybir.AluOpType

    TOTCH, CNT_TOT, NRCH = meta["TOTCH"], meta["CNT_TOT"], meta["NRCH"]
    AW, CW, STG = meta["AW"], meta["CW"], meta["STG"]

    nc = bacc.Bacc(None, num_devices=C)
    tab_x = nc.dram_tensor("tab", [_HDR, D], f32, kind="ExternalInput")
    wext_x = nc.dram_tensor("wext", [74, 2 * D], f32, kind="ExternalInput")
    iota_x = nc.dram_tensor("iota", [P, 512], f32, kind="ExternalInput")
    io16_x = nc.dram_tensor("iota16", [P, 16], f32, kind="ExternalInput")
    idxA_x = nc.dram_tensor("idxA", [P, AW], i16, kind="ExternalInput")
    idxC_x = nc.dram_tensor("idxC", [P, CW], i16, kind="ExternalInput")
    cdst_x = nc.dram_tensor("cdst", [P, TOTCH], f32, kind="ExternalInput")
    cnts_x = nc.dram_tensor("cnts", [10, CNT_TOT], f32, kind="ExternalInput")
    rgid_x = nc.dram_tensor("rgid", [P, NRCH], f32, kind="ExternalInput")
    out_x = nc.dram_tensor("out", [B, 2 * D], f32, kind="ExternalOutput")

    T = nc.dram_tensor("T", [meta["T_ROWS"], D], f32, addr_space="Shared")
    import os
    dump_T = bool(os.environ.get("KERNEL_DUMP_T"))
    tdump_x = (nc.dram_tensor("tdump", [meta["T_ROWS"], D], f32,
                              kind="ExternalOutput") if dump_T else None)
    agin = nc.dram_tensor("agin", [meta["AG_ROWS"], D], f32)
    stg_t = nc.dram_tensor("stg", [max(STG, 128), D], f32)
    prin = nc.dram_tensor("prin", [2 * B, D], f32)
    prout = nc.dram_tensor("prout", [C * 2 * B, D], f32, addr_space="Shared")

    rg = [list(range(C))]

    with tile.TileContext(nc) as tc:
        with (
            tc.tile_pool(name="const", bufs=1) as constp,
            tc.tile_pool(name="g", bufs=2) as gpool,
            tc.tile_pool(name="ro", bufs=1) as ropool,
            tc.tile_pool(name="sall", bufs=2) as sallp,
            tc.tile_pool(name="sx", bufs=4) as sxp,
            tc.tile_pool(name="lhs", bufs=2) as lhsp,
            tc.tile_pool(name="msg", bufs=2) as msgp,
            tc.tile_pool(name="small", bufs=2) as smp,
            tc.tile_pool(name="psA", bufs=2, space="PSUM") as psA,
            tc.tile_pool(name="psB", bufs=2, space="PSUM") as psB,
            tc.tile_pool(name="psC", bufs=2, space="PSUM") as psC,
            tc.tile_pool(name="psR", bufs=1, space="PSUM") as psR,
        ):
            # ---------------- setup ----------------
            stg = constp.tile([_HDR, D], f32)
            nc.sync.dma_start(out=stg[:, :], in_=tab_x[:, :])
            nc.sync.dma_start(out=T[0:_HDR, :], in_=stg[:, :])
            wt = constp.tile([74, 2 * D], f32)
            nc.sync.dma_start(out=wt[:, :], in_=wext_x[:, :])
            iot = constp.tile([P, 512], f32)
            nc.sync.dma_start(out=iot[:, :], in_=iota_x[:, :])
            io16 = constp.tile([P, 16], f32)
            nc.sync.dma_start(out=io16[:, :], in_=io16_x[:, :])
            cdstt = constp.tile([P, TOTCH], f32)
            nc.sync.dma_start(out=cdstt[:, :], in_=cdst_x[:, :])
            idxAt = constp.tile([P, AW], i16)
            nc.sync.dma_start(out=idxAt[:, :], in_=idxA_x[:, :])
            idxCt = constp.tile([P, CW], i16)
            nc.sync.dma_start(out=idxCt[:, :], in_=idxC_x[:, :])
            rgidt = constp.tile([P, NRCH], f32)
            nc.sync.dma_start(out=rgidt[:, :], in_=rgid_x[:, :])
            ident = constp.tile([P, P], f32)
            make_identity(nc, ident[:, :])

            def two_hop_gather(segs, A_pad, acol0, ccol0, stg0, n_slots,
                               gpool_, tagA, tagC, frontier):
                """Emit phase A (window gathers) -> staging -> phase C."""
                nchA = A_pad // P
                GA = gpool_.tile([P, nchA * D], f32, tag=tagA)
                base = 0
                for (swb, n) in segs:
                    wrows = min(_W, frontier - swb)
                    k = n // P
                    j0 = base // P
                    nc.gpsimd.dma_gather(
                        out_ap=GA[:, j0 * D:(j0 + k) * D].rearrange(
                            "p (j f) -> p j f", f=D),
                        in_ap=T[swb:swb + wrows, :],
                        idxs_ap=idxAt[:, acol0 + base // 16:
                                      acol0 + (base + n) // 16],
                        num_idxs=n, num_idxs_reg=n, elem_size=D,
                        single_packet=False)
                    base += n
                nc.sync.dma_start(
                    out=stg_t[stg0:stg0 + A_pad, :].rearrange(
                        "(j p) f -> p j f", p=P),
                    in_=GA[:, :].rearrange("p (j f) -> p j f", f=D))
                k = n_slots // P
                G = gpool_.tile([P, k * D], f32, tag=tagC)
                nc.gpsimd.dma_gather(
                    out_ap=G[:, :].rearrange("p (j f) -> p j f", f=D),
                    in_ap=stg_t[stg0:stg0 + A_pad, :],
                    idxs_ap=idxCt[:, ccol0:ccol0 + n_slots // 16],
                    num_idxs=n_slots, num_idxs_reg=n_slots, elem_size=D,
                    single_packet=False)
                return G

            # ---------------- level sweeps ----------------
            max_lv = int(os.environ.get("KERNEL_MAX_LEVELS", "99"))
            skip_ro = bool(os.environ.get("KERNEL_SKIP_READOUT"))
            for pl in meta["plans"][:max_lv]:
                Lp, ww, nch, nwin = pl.Lp, pl.ww, pl.nch, pl.nwin
                nw2 = Lp // P
                lhs = lhsp.tile([74, Lp], f32)
                nc.sync.dma_start(
                    out=lhs[64:74, :],
                    in_=cnts_x[:, pl.cnt0:pl.cnt0 + Lp])
                if nch > 0:
                    G = two_hop_gather(pl.segs, pl.A_pad, pl.acol0, pl.ccol0,
                                       pl.stg0, nch * P, gpool, "GA", "G",
                                       pl.tb)
                    # bulk k=0 selection matrices
                    Sall = sallp.tile([P, nch * ww], f32)
                    ia = iot[:, 0:ww]
                    iota_rep = bass.AP(
                        ia.tensor, ia.offset, [ia.ap[0], [0, nch], [1, ww]])
                    cdb = cdstt[:, pl.ch0:pl.ch0 + nch].to_broadcast(
                        [P, nch, ww])
                    nc.vector.tensor_tensor(
                        out=Sall[:, :].rearrange("p (a b) -> p a b", a=nch),
                        in0=iota_rep, in1=cdb, op=OP.is_equal)
                    extS = {}
                    for (j, k) in pl.extras:
                        S2 = sxp.tile([P, ww], f32)
                        nc.vector.tensor_scalar(
                            S2[:, :], iot[:, 0:ww],
                            cdstt[:, pl.ch0 + j:pl.ch0 + j + 1],
                            float(-k * ww), OP.subtract, OP.is_equal)
                        extS[(j, k)] = S2
                    for w in range(nwin):
                        width = min(ww, Lp - w * ww)
                        srcs = pl.win_srcs[w]
                        if not srcs:
                            nc.vector.memset(
                                lhs[0:64, w * ww:w * ww + width], 0.0)
                            continue
                        ps = psA.tile([64, ww], f32)
                        for i, (j, k) in enumerate(srcs):
                            if k == 0:
                                S_ap = Sall[:, j * ww:j * ww + width]
                            else:
                                S_ap = extS[(j, k)][:, 0:width]
                            nc.tensor.matmul(
                                out=ps[:, 0:width],
                                lhsT=G[:, j * D:(j + 1) * D],
                                rhs=S_ap,
                                start=(i == 0), stop=(i == len(srcs) - 1))
                        nc.vector.tensor_copy(
                            out=lhs[0:64, w * ww:w * ww + width],
                            in_=ps[:, 0:width])
                else:
                    nc.vector.memset(lhs[0:64, :], 0.0)

                msg = msgp.tile([P, nw2 * D], f32)
                wcol = 0 if pl.sweep == "f" else D
                for w2 in range(nw2):
                    ps2 = psB.tile([P, D], f32)
                    nc.tensor.matmul(
                        out=ps2[:, :],
                        lhsT=lhs[:, w2 * P:(w2 + 1) * P],
                        rhs=wt[:, wcol:wcol + D],
                        start=True, stop=True)
                    nc.vector.tensor_copy(
                        out=msg[:, w2 * D:(w2 + 1) * D], in_=ps2[:, :])
                nc.sync.dma_start(
                    out=agin[pl.ag0:pl.ag0 + Lp, :].rearrange(
                        "(j p) f -> p j f", p=P),
                    in_=msg[:, :].rearrange("p (j f) -> p j f", f=D))
                nc.gpsimd.collective_compute(
                    "AllGather", OP.bypass, replica_groups=rg,
                    ins=[agin[pl.ag0:pl.ag0 + Lp, :]],
                    outs=[T[pl.tb:pl.tb + C * Lp, :]])

            # ---------------- readout ----------------
            if skip_ro:
                zo = smp.tile([B, 2 * D], f32, tag="outt")
                nc.vector.memset(zo[:, :], 0.0)
                nc.sync.dma_start(out=out_x[:, :], in_=zo[:, :])
                if dump_T:
                    nc.sync.dma_start(out=tdump_x[:, :], in_=T[:, :])
            if not skip_ro:
                _emit_readout(
                    nc, bass, mybir, meta, tc, constp, ropool, sallp, smp,
                    psB, psC, psR, rgidt, io16, ident, stg_t, idxAt, idxCt,
                    agin, prin, prout, out_x, tdump_x, dump_T, two_hop_gather,
                    rg, T)
    nc.compile()
    return nc


def _emit_readout(nc, bass, mybir, meta, tc, constp, ropool, sallp, smp,
                  psB, psC, psR, rgidt, io16, ident, stg_t, idxAt, idxCt,
                  agin, prin, prout, out_x, tdump_x, dump_T, two_hop_gather,
                  rg, T):
    f32 = mybir.dt.float32
    AX = mybir.AxisListType
    OP = mybir.AluOpType
    NRCH = meta["NRCH"]
    import os
    stage = int(os.environ.get("KERNEL_RO_STAGE", "9"))
    if True:
        if True:
            ro = meta["ro"]
            R = two_hop_gather(ro["segs"], ro["A_pad"], ro["acol0"],
                               ro["ccol0"], ro["stg0"], NRCH * P,
                               ropool, "RA", "R", meta["T_ROWS"])
            if stage <= 1:
                zo = smp.tile([B, 2 * D], f32, tag="outt")
                nc.vector.tensor_copy(out=zo[:, :], in_=R[0:B, 0:2 * D])
                nc.sync.dma_start(out=out_x[:, :], in_=zo[:, :])
                return
            S16 = sallp.tile([P, NRCH * 16], f32, tag="s16")
            i16 = io16[:, 0:16]
            i16_rep = bass.AP(
                i16.tensor, i16.offset, [i16.ap[0], [0, NRCH], [1, 16]])
            rgb = rgidt[:, :].to_broadcast([P, NRCH, 16])
            nc.vector.tensor_tensor(
                out=S16[:, :].rearrange("p (a b) -> p a b", a=NRCH),
                in0=i16_rep, in1=rgb, op=OP.is_equal)
            ps_sum = psR.tile([B, D], f32)
            for j in range(NRCH):
                nc.tensor.matmul(
                    out=ps_sum[:, :],
                    lhsT=S16[:, j * 16:(j + 1) * 16],
                    rhs=R[:, j * D:(j + 1) * D],
                    start=(j == 0), stop=(j == NRCH - 1))
            if stage <= 2:
                zo = smp.tile([B, 2 * D], f32, tag="outt")
                nc.vector.tensor_copy(out=zo[:, 0:D], in_=ps_sum[:, :])
                nc.vector.memset(zo[:, D:2 * D], 0.0)
                nc.sync.dma_start(out=out_x[:, :], in_=zo[:, :])
                return
            maxT = constp.tile([64, B], f32)
            for g, (c0, kg) in enumerate(meta["graph_chunks"]):
                if kg == 1:
                    mx_ap = R[:, c0 * D:(c0 + 1) * D]
                else:
                    mx = smp.tile([P, D], f32, tag="mx")
                    nc.vector.tensor_tensor(
                        out=mx[:, :], in0=R[:, c0 * D:(c0 + 1) * D],
                        in1=R[:, (c0 + 1) * D:(c0 + 2) * D], op=OP.max)
                    for q in range(2, kg):
                        nc.vector.tensor_tensor(
                            out=mx[:, :], in0=mx[:, :],
                            in1=R[:, (c0 + q) * D:(c0 + q + 1) * D],
                            op=OP.max)
                    mx_ap = mx[:, :]
                pst = psC.tile([64, P], f32, tag="pst")
                nc.tensor.transpose(
                    out=pst[:, :], in_=mx_ap, identity=ident[:, :])
                nc.vector.reduce_max(
                    out=maxT[:, g:g + 1], in_=pst[:, :], axis=AX.X)
            if stage <= 3:
                zo = smp.tile([B, 2 * D], f32, tag="outt")
                nc.vector.memset(zo[:, :], 0.0)
                nc.sync.dma_start(out=out_x[:, :], in_=zo[:, :])
                return
            psmx = psC.tile([B, 64], f32, tag="pst")
            nc.tensor.transpose(
                out=psmx[:, :], in_=maxT[:, :], identity=ident[0:64, 0:64])
            pr = smp.tile([B, 2 * D], f32, tag="pr")
            nc.vector.tensor_copy(out=pr[:, 0:D], in_=psmx[:, :])
            nc.vector.tensor_copy(out=pr[:, D:2 * D], in_=ps_sum[:, :])
            nc.sync.dma_start(
                out=prin[:, :].rearrange("(h g) f -> g h f", g=B),
                in_=pr[:, :].rearrange("g (h f) -> g h f", h=2))
            nc.gpsimd.collective_compute(
                "AllGather", OP.bypass, replica_groups=rg,
                ins=[prin[:, :]], outs=[prout[:, :]])
            if stage <= 4:
                zo = smp.tile([B, 2 * D], f32, tag="outt")
                nc.vector.memset(zo[:, :], 0.0)
                nc.sync.dma_start(out=out_x[:, :], in_=zo[:, :])
                return
            pr3 = prout[:, :].rearrange("(r gg) f -> gg r f", r=C)
            mx8 = smp.tile([B, C * D], f32, tag="mx8")
            sm8 = smp.tile([B, C * D], f32, tag="sm8")
            nc.sync.dma_start(
                out=mx8[:, :].rearrange("g (r f) -> g r f", f=D),
                in_=pr3[0:B])
            nc.sync.dma_start(
                out=sm8[:, :].rearrange("g (r f) -> g r f", f=D),
                in_=pr3[B:2 * B])
            outt = smp.tile([B, 2 * D], f32, tag="outt")
            t1m = smp.tile([B, 4 * D], f32, tag="t1m")
            t2m = smp.tile([B, 2 * D], f32, tag="t2m")
            nc.vector.tensor_tensor(
                out=t1m[:, :], in0=mx8[:, 0:4 * D], in1=mx8[:, 4 * D:8 * D],
                op=OP.max)
            nc.vector.tensor_tensor(
                out=t2m[:, :], in0=t1m[:, 0:2 * D], in1=t1m[:, 2 * D:4 * D],
                op=OP.max)
            nc.vector.tensor_tensor(
                out=outt[:, 0:D], in0=t2m[:, 0:D], in1=t2m[:, D:2 * D],
                op=OP.max)
            t1s = smp.tile([B, 4 * D], f32, tag="t1s")
            t2s = smp.tile([B, 2 * D], f32, tag="t2s")
            nc.vector.tensor_tensor(
                out=t1s[:, :], in0=sm8[:, 0:4 * D], in1=sm8[:, 4 * D:8 * D],
                op=OP.add)
            nc.vector.tensor_tensor(
                out=t2s[:, :], in0=t1s[:, 0:2 * D], in1=t1s[:, 2 * D:4 * D],
                op=OP.add)
            nc.vector.tensor_tensor(
                out=outt[:, D:2 * D], in0=t2s[:, 0:D], in1=t2s[:, D:2 * D],
                op=OP.add)
            nc.sync.dma_start(out=out_x[:, :], in_=outt[:, :])
            if dump_T:
                nc.sync.dma_start(out=tdump_x[:, :], in_=T[:, :])


def _in_maps(meta, arrays):
    maps = []
    for c in range(C):
        maps.append(dict(
            tab=arrays["tab"],
            wext=arrays["wext"],
            iota=arrays["iota512"],
            iota16=arrays["iota16"],
            idxA=np.ascontiguousarray(arrays["idxA"][c]),
            idxC=np.ascontiguousarray(arrays["idxC"][c]),
            cdst=np.ascontiguousarray(arrays["cdst"][c]),
            cnts=np.ascontiguousarray(arrays["cnts"][c]),
            rgid=np.ascontiguousarray(arrays["rgid"][c]),
        ))
    return maps


_LAST_RESULTS = None  # stash for test harness (exec time, trace)


def kernel(**inputs):
    global _LAST_RESULTS
    import os
    meta, arrays = _preprocess(**inputs)
    nc = _build(meta)
    from concourse.bass_utils import run_bass_kernel_spmd
    res = run_bass_kernel_spmd(nc, _in_maps(meta, arrays),
                               core_ids=list(range(C)),
                               trace=bool(os.environ.get("KERNEL_TRACE")))
    _LAST_RESULTS = res
    return np.asarray(res.results[0]["out"])



# revision 29
# speedup vs baseline: 1.5051x; 1.5051x over previous
"""DAG-GNN level-sweep kernel for Trainium2 (8 NeuronCores, Bass/Tile), v2.

Design vs v1:
  - State table T in DRAM is fp16 (tolerance 2e-2 >> fp16's ~3e-4 here);
    gathers fetch PAIRS of consecutive node rows (256B) so the int16
    dma_gather index window covers 65536 node rows, and each level's
    sources fit in a handful of windows.
  - ONE-hop gather per level (no staging round-trip, no permutation
    gather): slots sorted by (window, parity, dst col); chunks are
    window- and parity-pure; the per-chunk selection matmul picks the
    even or odd half of each gathered pair.
  - Old/fresh split: gathers whose sources predate the previous level's
    AllGather are emitted one level early so they overlap the previous
    level's compute + collective; only the small "fresh" gather (sources
    in the immediately preceding level block) sits on the critical path.
  - AllGather moves fp16 (half the bytes of v1).
  - Readout keeps a two-hop (phase A windows -> staging -> graph/parity
    grouped chunks) but phase A overlaps the last levels.
"""

import sys

if "/opt/trn_rl_repo" not in sys.path:
    sys.path.insert(0, "/opt/trn_rl_repo")

import numpy as np

# structural constants of the nn.Module (match reference)
B = 16   # graphs per batch
LF = 12  # forward topological levels
LB = 12  # backward topological levels
D = 64   # hidden dim
C = 8    # NeuronCores
P = 128  # SBUF partitions

# T table header rows (node-row space; pairs are rows (2q, 2q+1))
_H0 = 1        # 9 rows of initial-encoding table, indexed by 3*nt + nip
_MI0 = 10      # fp16-min row, even parity
_MI1 = 11      # fp16-min row, odd parity
_HDR = 12

_WPAIR = 32768   # pair rows addressable by one int16 gather window
_F16MIN = -65504.0


def _ceil(a, b):
    return -(-a // b)


_SPAN = 512  # bulk selection-matrix width per chunk (max matmul width)


class _Lvl:
    __slots__ = (
        "sweep", "l", "Lp", "tb", "nw2", "cnt0",
        "calls", "nch", "ch0", "nch_old", "c0", "pieces",
    )


class _Call:
    """One dma_gather call: window [base_pair, base_pair+ext_pair) of T."""
    __slots__ = ("base_pair", "ext_pair", "icol0", "n", "fresh")

    def __init__(self, base_pair, ext_pair, icol0, n, fresh):
        self.base_pair = base_pair
        self.ext_pair = ext_pair
        self.icol0 = icol0
        self.n = n
        self.fresh = fresh


def _wrap_idx(a):
    """[C, n] int -> [C, 128, n/16] int16 (16-partition wrap, 8 replicas)."""
    Csz, n = a.shape
    w = a.reshape(Csz, n // 16, 16).transpose(0, 2, 1).astype(np.int16)
    return np.ascontiguousarray(np.tile(w, (1, 8, 1)))


def _preprocess(node_type, num_inverted_predecessors, edge_index,
                forward_level, backward_level, batch,
                W_enc, b_enc, W_f, b_f, W_b, b_b):
    N = int(node_type.shape[0])
    nt = np.asarray(node_type).astype(np.int64)
    nip = np.asarray(num_inverted_predecessors).astype(np.int64)
    fl = np.asarray(forward_level).astype(np.int64)
    bl = np.asarray(backward_level).astype(np.int64)
    bt = np.asarray(batch).astype(np.int64)
    src = np.asarray(edge_index[0]).astype(np.int64)
    dst = np.asarray(edge_index[1]).astype(np.int64)
    code = nt * 3 + nip  # in [0, 9)

    # ---------------- node positions in T ----------------
    posf = np.full(N, -1, np.int64)
    posb = np.full(N, -1, np.int64)
    coref = np.zeros(N, np.int64)
    rankf = np.zeros(N, np.int64)
    coreb = np.zeros(N, np.int64)
    rankb = np.zeros(N, np.int64)

    tbase = _HDR
    lvl_meta = []
    for sweep, lv, pos, core, rank, nl in (
        ("f", fl, posf, coref, rankf, LF),
        ("b", bl, posb, coreb, rankb, LB),
    ):
        for l in range(1, nl):
            idx = np.flatnonzero(lv == l)
            n_l = idx.size
            if n_l == 0:
                lvl_meta.append((sweep, l, 0, tbase))
                continue
            Lp = _ceil(_ceil(n_l, C), P) * P
            c = np.arange(n_l) % C
            r = np.arange(n_l) // C
            pos[idx] = tbase + c * Lp + r
            core[idx] = c
            rank[idx] = r
            lvl_meta.append((sweep, l, Lp, tbase))
            tbase += C * Lp
    T_ROWS = tbase

    # ---------------- per-level plans ----------------
    plans = []
    cdst_blocks = []
    prt_blocks = []
    cnts_blocks = []
    idx_cols = []      # list of [C, 128, k] int16 blocks
    icol0 = 0
    cnt0 = 0
    ch0 = 0
    fl_dst = fl[dst]
    bl_src = bl[src]
    prev_block = None  # (tb, rows) of previous plan's T block
    for (sweep, l, Lp, tb) in lvl_meta:
        if Lp == 0:
            continue
        pl = _Lvl()
        pl.sweep, pl.l, pl.Lp, pl.tb = sweep, l, Lp, tb
        pl.nw2 = Lp // P
        pl.cnt0 = cnt0
        if sweep == "f":
            em = fl_dst == l
            un = dst[em]
            dn = src[em]
            ucore = coref[un]
            urank = rankf[un]
            gat = (fl[dn] >= 1) & (fl[dn] < l)
            gpos_all = posf[dn]
        else:
            em = bl_src == l
            un = src[em]
            dn = dst[em]
            ucore = coreb[un]
            urank = rankb[un]
            upd_b = (bl[dn] >= 1) & (bl[dn] < l)
            upd_f = (~upd_b) & (fl[dn] >= 1)
            gat = upd_b | upd_f
            gpos_all = np.where(upd_b, posb[dn], posf[dn])

        # counts: initial-valued sources by code, plus total indegree (bias)
        cnt = np.zeros((C, Lp, 10), np.float32)
        i0 = ~gat
        np.add.at(cnt, (ucore[i0], urank[i0], code[dn[i0]]), 1.0)
        np.add.at(cnt, (ucore, urank, 9), 1.0)
        cnts_blocks.append(np.ascontiguousarray(
            cnt.transpose(0, 2, 1)).astype(np.float16))
        cnt0 += Lp

        gc = ucore[gat]
        gr = urank[gat]        # dst col (rank == col)
        gp = gpos_all[gat]     # source node row in T
        if gp.size == 0:
            pl.calls, pl.nch, pl.ch0 = [], 0, ch0
            pl.nch_old, pl.c0, pl.pieces = 0, [], []
            plans.append(pl)
            prev_block = (tb, C * Lp)
            continue

        # window id: old region [0, fs) in _WPAIR-pair windows; fresh = prev
        fs = prev_block[0]
        fresh_rows = prev_block[1]
        n_oldwin = _ceil(fs // 2, _WPAIR) if fs > 0 else 0
        wid = np.where(gp < fs, (gp // 2) // _WPAIR, n_oldwin)
        par = gp % 2
        poff = np.where(gp < fs, (gp // 2) % _WPAIR, gp // 2 - fs // 2)

        # group sizes per window (parity folded into the mask): uniform
        gsz = np.zeros(n_oldwin + 1, np.int64)
        for w in range(n_oldwin + 1):
            mx = 0
            for c in range(C):
                mx = max(mx, int(((gc == c) & (wid == w)).sum()))
            gsz[w] = _ceil(mx, P) * P if mx else 0
        S_tot = int(gsz.sum())
        nch = S_tot // P

        # slot arrays [C, S_tot]
        off_sl = np.zeros((C, S_tot), np.int64)   # pair offset in window
        col_sl = np.full((C, S_tot), -1, np.int64)  # dst col (-1 padding)
        par_sl = np.zeros((C, S_tot), np.int64)
        gstart = np.concatenate([[0], np.cumsum(gsz)])[:-1]
        for c in range(C):
            m_c = gc == c
            order = np.lexsort((gr[m_c], wid[m_c]))
            wo = wid[m_c][order]
            oo = poff[m_c][order]
            ro = gr[m_c][order]
            qo = par[m_c][order]
            cnts_g = np.bincount(wo, minlength=n_oldwin + 1)
            st = np.concatenate([[0], np.cumsum(cnts_g)])[:-1]
            seq = np.arange(wo.size) - st[wo]
            slot = gstart[wo] + seq
            off_sl[c, slot] = oo
            col_sl[c, slot] = ro
            par_sl[c, slot] = qo

        # gather calls: one per window with slots
        calls = []
        nch_old = 0
        for w in range(n_oldwin + 1):
            n_g = int(gsz[w])
            if n_g == 0:
                continue
            if w < n_oldwin:
                base = w * _WPAIR
                ext = min(_WPAIR, fs // 2 - base)
                fresh = False
                nch_old += n_g // P
            else:
                base = fs // 2
                ext = fresh_rows // 2
                fresh = True
            a = int(gstart[w])
            blk = off_sl[:, a:a + n_g]
            idx_cols.append(_wrap_idx(blk))
            calls.append(_Call(base, ext, icol0, n_g, fresh))
            icol0 += n_g // 16
        pl.calls = calls
        pl.nch_old = nch_old

        # per-chunk base col + exact-span pieces; cdst = col - c0_j
        c0 = np.zeros(nch, np.int64)
        pieces = []
        for j in range(nch):
            v = col_sl[:, j * P:(j + 1) * P]
            vv = v[v >= 0]
            lo = int(vv.min()) if vv.size else 0
            hi = int(vv.max()) if vv.size else 0
            c0[j] = lo
            span = hi - lo + 1
            pc = []
            k = 0
            while span > 0:
                w_k = min(_SPAN, span)
                pc.append((k, w_k))
                span -= w_k
                k += 1
            pieces.append(pc)
        cd = col_sl.astype(np.float64) - c0[np.arange(S_tot) // P][None, :]
        cd[col_sl < 0] = -1.0
        cdst_blocks.append(np.ascontiguousarray(
            cd.reshape(C, nch, P).transpose(0, 2, 1)).astype(np.float32))
        prt_blocks.append(np.ascontiguousarray(
            par_sl.reshape(C, nch, P).transpose(0, 2, 1)).astype(np.float32))
        pl.nch, pl.ch0 = nch, ch0
        pl.c0 = c0.tolist()
        pl.pieces = pieces
        ch0 += nch
        plans.append(pl)
        prev_block = (tb, C * Lp)

    TOTCH = max(1, ch0)
    CNT_TOT = cnt0
    cdst_all = (np.concatenate(cdst_blocks, axis=2) if cdst_blocks
                else np.full((C, P, 1), -1.0, np.float32))
    prt_all = (np.concatenate(prt_blocks, axis=2) if prt_blocks
               else np.zeros((C, P, 1), np.float32))
    if cdst_all.shape[2] < TOTCH:
        pad = TOTCH - cdst_all.shape[2]
        cdst_all = np.concatenate(
            [cdst_all, np.full((C, P, pad), -1.0, np.float32)], axis=2)
        prt_all = np.concatenate(
            [prt_all, np.zeros((C, P, pad), np.float32)], axis=2)
    cnts_all = np.concatenate(cnts_blocks, axis=2)

    # ---------------- readout layout ----------------
    last_tb = plans[-1].tb
    last_rows = C * plans[-1].Lp
    mid_tb = plans[-2].tb
    onodes = np.flatnonzero(nt == 1)
    og = bt[onodes]
    fpos = np.where(bl[onodes] >= 1, posb[onodes],
                    np.where(fl[onodes] >= 1, posf[onodes],
                             _H0 + code[onodes]))
    oc = np.arange(onodes.size) % C  # core assignment

    # --- phase A: slots grouped per window; stage order = arrival order
    # stage 0: rows < mid_tb (oldest); 1: [mid_tb, last_tb); 2: last block
    n_oldwinA = _ceil(mid_tb // 2, _WPAIR)
    widA = np.where(fpos < mid_tb, (fpos // 2) // _WPAIR,
                    np.where(fpos < last_tb, n_oldwinA, n_oldwinA + 1))
    basesA = [w * _WPAIR for w in range(n_oldwinA)] + \
        [mid_tb // 2, last_tb // 2]
    extsA = [min(_WPAIR, mid_tb // 2 - w * _WPAIR)
             for w in range(n_oldwinA)] + \
        [(last_tb - mid_tb) // 2, last_rows // 2]
    stageA = [0] * n_oldwinA + [1, 2]
    nwinA = n_oldwinA + 2
    # per (window) sizes uniform across cores; slot 0 of window 0 reserved
    # for the fp16-min pair (pair row 5) used as padding target.
    poffA = fpos // 2 - np.asarray(basesA, np.int64)[widA]
    gszA = np.zeros(nwinA, np.int64)
    for w in range(nwinA):
        mx = 0
        for c in range(C):
            mx = max(mx, int(((oc == c) & (widA == w)).sum()))
        mx += (1 if w == 0 else 0)  # reserved pad slot
        gszA[w] = _ceil(mx, P) * P if mx else 0
    A_tot = int(gszA.sum())
    kA = A_tot // P
    gstartA = np.concatenate([[0], np.cumsum(gszA)])[:-1]
    offA = np.zeros((C, A_tot), np.int64)
    offA[:, 0] = _MI0 // 2  # pad pair (both halves fp16-min)
    # staged row of phase-A slot q (GA[p, j] -> stg row p*kA+j)
    stg_row = (np.arange(A_tot) % P) * kA + np.arange(A_tot) // P
    posA = np.zeros((C, onodes.size), np.int64)  # node -> phase-A slot
    for c in range(C):
        m_c = oc == c
        order = np.argsort(widA[m_c], kind="stable")
        wo = widA[m_c][order]
        oo = poffA[m_c][order]
        cnts_g = np.bincount(wo, minlength=nwinA)
        st = np.concatenate([[0], np.cumsum(cnts_g)])[:-1]
        seq = np.arange(wo.size) - st[wo]
        slot = gstartA[wo] + seq + (wo == 0)  # skip reserved slot 0
        offA[c, slot] = oo
        posA[c, np.flatnonzero(m_c)[order]] = slot
    ro_callsA = []
    icolA_blocks = []
    for w in range(nwinA):
        n_g = int(gszA[w])
        if n_g == 0:
            continue
        a = int(gstartA[w])
        icolA_blocks.append(offA[:, a:a + n_g])
        ro_callsA.append(_Call(basesA[w], extsA[w], icol0, n_g, stageA[w]))
        icol0 += n_g // 16
    for blk in icolA_blocks:
        idx_cols.append(_wrap_idx(blk))

    # --- phase C: final chunks grouped by (graph, parity)
    parR = fpos % 2
    kg2 = np.zeros((B, 2), np.int64)
    for g in range(B):
        for q in range(2):
            mx = 0
            for c in range(C):
                mx = max(mx, int(((oc == c) & (og == g) &
                                  (parR == q)).sum()))
            kg2[g, q] = _ceil(max(mx, 1), P) * P
    R_tot = int(kg2.sum())
    NRCH = R_tot // P
    gstartR = np.concatenate([[0], np.cumsum(kg2.reshape(-1))])[:-1]
    gstartR = gstartR.reshape(B, 2)
    idxC = np.zeros((C, R_tot), np.int64)
    idxC[:, :] = int(stg_row[0])  # padding -> staged fp16-min pair
    rgid = np.full((C, R_tot), -1.0, np.float32)
    chunk_parR = np.zeros(NRCH, np.int64)
    graph_chunks = []  # per graph: list of chunk ids
    for g in range(B):
        chs = []
        for q in range(2):
            a, n_g = int(gstartR[g, q]), int(kg2[g, q])
            chunk_parR[a // P:(a + n_g) // P] = q
            chs.extend(range(a // P, (a + n_g) // P))
        graph_chunks.append(chs)
    for c in range(C):
        m_c = oc == c
        order = np.lexsort((parR[m_c], og[m_c]))
        go = og[m_c][order]
        qo = parR[m_c][order]
        so = posA[c, m_c][order]
        grp = go * 2 + qo
        cnts_g = np.bincount(grp, minlength=B * 2)
        st = np.concatenate([[0], np.cumsum(cnts_g)])[:-1]
        seq = np.arange(go.size) - st[grp]
        slot = gstartR[go, qo] + seq
        idxC[c, slot] = stg_row[so]
        rgid[c, slot] = go
    icolC = icol0
    idx_cols.append(_wrap_idx(idxC))
    icol0 += R_tot // 16
    rgid_arr = np.ascontiguousarray(
        rgid.reshape(C, NRCH, P).transpose(0, 2, 1))

    # ---------------- weight-derived constants ----------------
    W_enc = np.asarray(W_enc, np.float32)
    b_enc = np.asarray(b_enc, np.float32)
    W_f = np.asarray(W_f, np.float32)
    b_f = np.asarray(b_f, np.float32)
    W_b = np.asarray(W_b, np.float32)
    b_b = np.asarray(b_b, np.float32)
    h0_tab = np.zeros((9, D), np.float32)
    for cc in range(9):
        h0_tab[cc] = (cc // 3) * W_enc[0] + (cc % 3) * W_enc[1] + b_enc
    tab = np.zeros((_HDR, D), np.float32)
    tab[_H0:_H0 + 9] = h0_tab
    tab[_MI0] = _F16MIN
    tab[_MI1] = _F16MIN
    wf_ext = np.concatenate([W_f, h0_tab @ W_f, b_f[None, :]], axis=0)
    wb_ext = np.concatenate([W_b, h0_tab @ W_b, b_b[None, :]], axis=0)
    wext = np.ascontiguousarray(np.concatenate([wf_ext, wb_ext], axis=1))

    iota512 = np.ascontiguousarray(
        np.tile(np.arange(512, dtype=np.float32), (P, 1)))
    iota16 = np.ascontiguousarray(
        np.tile(np.arange(16, dtype=np.float32), (P, 1)))

    idxA_all = np.concatenate(idx_cols, axis=2)
    meta = dict(
        plans=plans, graph_chunks=graph_chunks,
        T_ROWS=T_ROWS, AG_ROWS=max(1, CNT_TOT), TOTCH=TOTCH,
        CNT_TOT=max(1, CNT_TOT), NRCH=NRCH, AW=idxA_all.shape[2],
        ro=dict(callsA=ro_callsA, kA=kA, A_tot=A_tot, icolC=icolC,
                chunk_par=chunk_parR.tolist()),
    )
    arrays = dict(
        tab=tab.astype(np.float16), wext=wext.astype(np.float16),
        iota512=iota512, iota16=iota16, idxA=idxA_all,
        cdst=cdst_all, prt=prt_all, cnts=cnts_all, rgid=rgid_arr,
    )
    return meta, arrays


# ---------------------------------------------------------------------------
# pure-numpy execution of the plan (host self-check / debugging)
# ---------------------------------------------------------------------------

def _sim_gather(T16, arrays, c, calls, n_slots):
    """numpy mirror of the device one-hop pair gather -> [n_slots, 2, D]."""
    idxA = arrays["idxA"][c][0:16, :]
    out = np.zeros((n_slots, 2, D), np.float32)
    Tp = T16.reshape(-1, 2, D)
    base = 0
    for cl in calls:
        cols = slice(cl.icol0, cl.icol0 + cl.n // 16)
        off = idxA[:, cols].T.reshape(-1)[:cl.n].astype(np.int64)
        assert off.max(initial=0) < cl.ext_pair
        out[base:base + cl.n] = Tp[cl.base_pair + off]
        base += cl.n
    assert base == n_slots
    return out


def _simulate_plan(meta, arrays, fp16=True, return_T=False):
    def rnd(x):
        return (np.clip(x, _F16MIN, -_F16MIN).astype(np.float16)
                .astype(np.float32) if fp16 else x)

    T = np.zeros((meta["T_ROWS"], D), np.float32)
    T[0:_HDR] = arrays["tab"].astype(np.float32)
    wext = arrays["wext"].astype(np.float32)
    for pl in meta["plans"]:
        wmat = wext[:, 0:D] if pl.sweep == "f" else wext[:, D:2 * D]
        blocks = []
        for c in range(C):
            lhs = np.zeros((74, pl.Lp), np.float32)
            lhs[64:74] = arrays["cnts"][c, :, pl.cnt0:pl.cnt0 + pl.Lp]
            if pl.nch > 0:
                cdv = arrays["cdst"][c][:, pl.ch0:pl.ch0 + pl.nch]
                prv = arrays["prt"][c][:, pl.ch0:pl.ch0 + pl.nch]
                Gp = _sim_gather(T, arrays, c, pl.calls, pl.nch * P)
                # G[p, j, 2*D] = slot j*P+p (full pair), parity-masked
                G = Gp.reshape(pl.nch, P, 2 * D).transpose(1, 0, 2).copy()
                half = (np.arange(2 * D) // D)[None, None, :]
                G *= (prv[:, :, None] == half)
                ps = np.zeros((2 * D, pl.Lp), np.float32)
                for j in range(pl.nch):
                    for (k, w_k) in pl.pieces[j]:
                        a = pl.c0[j] + k * _SPAN
                        S = (cdv[:, j:j + 1] ==
                             (np.arange(w_k) + k * _SPAN)[None, :])
                        ps[:, a:a + w_k] += G[:, j, :].T @ S.astype(
                            np.float32)
                lhs[0:64] = rnd(ps[0:D] + ps[D:2 * D])
            blocks.append(rnd(lhs.T @ wmat))  # [Lp, D]
        T[pl.tb:pl.tb + C * pl.Lp] = np.concatenate(blocks, axis=0)
    # readout
    ro = meta["ro"]
    NRCH = meta["NRCH"]
    maxp = np.full((B, D), _F16MIN, np.float32)
    sump = np.zeros((B, D), np.float32)
    for c in range(C):
        GA = _sim_gather(T, arrays, c, ro["callsA"], ro["A_tot"])
        # staged row of slot q = (q % P) * kA + q // P
        stg = np.zeros((ro["A_tot"], 2, D), np.float32)
        q = np.arange(ro["A_tot"])
        stg[(q % P) * ro["kA"] + q // P] = GA
        idxC = arrays["idxA"][c][0:16,
                                 ro["icolC"]:ro["icolC"] + NRCH * P // 16]
        off = idxC.T.reshape(-1).astype(np.int64)
        R_lin = stg[off]  # [NRCH*P, 2, D]
        R = R_lin.reshape(NRCH, P, 2, D)
        gid = arrays["rgid"][c]  # [P, NRCH]
        for g, chs in enumerate(meta["graph_chunks"]):
            for j in chs:
                sl = R[j, :, ro["chunk_par"][j], :]  # [P, D]
                maxp[g] = np.maximum(maxp[g], sl.max(axis=0))
                msk = (gid[:, j] == g).astype(np.float32)
                sump[g] += msk @ sl
    out = np.concatenate([maxp, sump], axis=1)
    return (out, T) if return_T else out


# ---------------------------------------------------------------------------
# Bass program
# ---------------------------------------------------------------------------

def _build(meta):
    import os

    import concourse.bass as bass
    import concourse.mybir as mybir
    from concourse import bacc, tile
    from concourse.masks import make_identity

    f32 = mybir.dt.float32
    f16 = mybir.dt.float16
    i16 = mybir.dt.int16
    AX = mybir.AxisListType
    OP = mybir.AluOpType

    TOTCH, CNT_TOT, NRCH = meta["TOTCH"], meta["CNT_TOT"], meta["NRCH"]
    AW = meta["AW"]
    plans = meta["plans"]
    NQ = 4  # SWDGE queues, round-robin over gather calls

    nc = bacc.Bacc(None, num_devices=C, num_swdge_queues=NQ)
    tab_x = nc.dram_tensor("tab", [_HDR, D], f16, kind="ExternalInput")
    wext_x = nc.dram_tensor("wext", [74, 2 * D], f16, kind="ExternalInput")
    iota_x = nc.dram_tensor("iota", [P, 512], f32, kind="ExternalInput")
    io16_x = nc.dram_tensor("iota16", [P, 16], f32, kind="ExternalInput")
    idxA_x = nc.dram_tensor("idxA", [P, AW], i16, kind="ExternalInput")
    cdst_x = nc.dram_tensor("cdst", [P, TOTCH], f32, kind="ExternalInput")
    prt_x = nc.dram_tensor("prt", [P, TOTCH], f32, kind="ExternalInput")
    cnts_x = nc.dram_tensor("cnts", [10, CNT_TOT], f16, kind="ExternalInput")
    rgid_x = nc.dram_tensor("rgid", [P, NRCH], f32, kind="ExternalInput")
    out_x = nc.dram_tensor("out", [B, 2 * D], f32, kind="ExternalOutput")

    T = nc.dram_tensor("T", [meta["T_ROWS"], D], f16, addr_space="Shared")
    dump_T = bool(os.environ.get("KERNEL_DUMP_T"))
    tdump_x = (nc.dram_tensor("tdump", [meta["T_ROWS"], D], f16,
                              kind="ExternalOutput") if dump_T else None)
    agin = nc.dram_tensor("agin", [meta["AG_ROWS"], D], f16)
    ro = meta["ro"]
    stg_t = nc.dram_tensor("stg", [max(ro["A_tot"], P), 2 * D], f16)
    prin = nc.dram_tensor("prin", [2 * B, D], f32)
    prout = nc.dram_tensor("prout", [C * 2 * B, D], f32, addr_space="Shared")

    rg = [list(range(C))]
    qctr = [0]

    max_lv = int(os.environ.get("KERNEL_MAX_LEVELS", "99"))
    skip_ro = bool(os.environ.get("KERNEL_SKIP_READOUT"))
    plans = plans[:max_lv]
    NP_ = len(plans)

    with tile.TileContext(nc) as tc:
        with (
            tc.tile_pool(name="const", bufs=1) as constp,
            tc.tile_pool(name="g", bufs=2) as gpool,
            tc.tile_pool(name="ro", bufs=1) as ropool,
            tc.tile_pool(name="sall", bufs=2) as sallp,
            tc.tile_pool(name="sx", bufs=4) as sxp,
            tc.tile_pool(name="lhs", bufs=2) as lhsp,
            tc.tile_pool(name="msg", bufs=2) as msgp,
            tc.tile_pool(name="small", bufs=1) as smp,
            tc.tile_pool(name="psA", bufs=1, space="PSUM") as psA,
            tc.tile_pool(name="psB", bufs=2, space="PSUM") as psB,
            tc.tile_pool(name="psR", bufs=1, space="PSUM") as psR,
        ):
            psC = psA  # readout transposes reuse the (dead) psA slot
            # ---------------- setup ----------------
            stg0t = constp.tile([_HDR, D], f16)
            nc.sync.dma_start(out=stg0t[:, :], in_=tab_x[:, :])
            nc.sync.dma_start(out=T[0:_HDR, :], in_=stg0t[:, :])
            wt = constp.tile([74, 2 * D], f16)
            nc.sync.dma_start(out=wt[:, :], in_=wext_x[:, :])
            iot = constp.tile([P, 512], f32)
            nc.sync.dma_start(out=iot[:, :], in_=iota_x[:, :])
            io16 = constp.tile([P, 16], f32)
            nc.sync.dma_start(out=io16[:, :], in_=io16_x[:, :])
            cdstt = constp.tile([P, TOTCH], f32)
            nc.sync.dma_start(out=cdstt[:, :], in_=cdst_x[:, :])
            prtt = constp.tile([P, TOTCH], f32)
            nc.sync.dma_start(out=prtt[:, :], in_=prt_x[:, :])
            hiota = constp.tile([P, 2 * D], f32)
            nc.vector.tensor_scalar(
                hiota[:, :], iot[:, 0:2 * D], float(D), None, OP.is_ge)
            idxAt = constp.tile([P, AW], i16)
            nc.sync.dma_start(out=idxAt[:, :], in_=idxA_x[:, :])
            rgidt = constp.tile([P, NRCH], f32)
            nc.sync.dma_start(out=rgidt[:, :], in_=rgid_x[:, :])
            ident = constp.tile([P, P], f32)
            make_identity(nc, ident[:, :])

            def emit_gather(calls, which, G, frontier):
                """Emit dma_gather calls; which selects fresh/old/all."""
                base = 0
                for cl in calls:
                    n = cl.n
                    j0 = base // P
                    k = n // P
                    base += n
                    if which == "fresh" and not cl.fresh:
                        continue
                    if which == "old" and cl.fresh:
                        continue
                    wrows = min(cl.ext_pair, frontier // 2 - cl.base_pair)
                    q = qctr[0] % NQ
                    qctr[0] += 1
                    nc.gpsimd.dma_gather(
                        out_ap=G[:, j0 * 2 * D:(j0 + k) * 2 * D].rearrange(
                            "p (j f) -> p j f", f=2 * D),
                        in_ap=T[cl.base_pair * 2:
                                (cl.base_pair + wrows) * 2, :].rearrange(
                            "(q x) f -> q (x f)", x=2),
                        idxs_ap=idxAt[:, cl.icol0:cl.icol0 + n // 16],
                        num_idxs=n, num_idxs_reg=n, elem_size=2 * D,
                        single_packet=False, queue_num=q)

            state = {}

            def emit_aggs(pl, st, j_range):
                G, ps, Sall = st["G"], st["ps"], st["Sall"]
                for j in j_range:
                    for (k, w_k) in pl.pieces[j]:
                        a = pl.c0[j] + k * _SPAN
                        if k == 0:
                            S_ap = Sall[:, j * _SPAN:j * _SPAN + w_k]
                        else:
                            S2 = sxp.tile([P, _SPAN], f16, tag="S2")
                            nc.vector.tensor_scalar(
                                S2[:, 0:w_k], iot[:, 0:w_k],
                                cdstt[:, pl.ch0 + j:pl.ch0 + j + 1],
                                float(-k * _SPAN), OP.subtract, OP.is_equal)
                            S_ap = S2[:, 0:w_k]
                        nc.tensor.matmul(
                            out=ps[:, a:a + w_k],
                            lhsT=G[:, j * 2 * D:(j + 1) * 2 * D],
                            rhs=S_ap,
                            start=False, stop=False, skip_group_check=True)

            def emit_prep(i):
                """S build (DVE), counts DMA, old gathers, old aggs."""
                pl = plans[i]
                Lp, nch = pl.Lp, pl.nch
                lhs = lhsp.tile([74, Lp], f16, tag="lhs")
                nc.sync.dma_start(
                    out=lhs[64:74, :],
                    in_=cnts_x[:, pl.cnt0:pl.cnt0 + Lp])
                st = dict(lhs=lhs)
                if nch > 0:
                    G = gpool.tile([P, nch * 2 * D], f16, tag="G")
                    st["G"] = G
                    emit_gather(pl.calls, "old", G, meta["T_ROWS"] * 2)
                    # parity masks for all chunks + selection matrices
                    M = sallp.tile([P, nch * 2 * D], f16, tag="M")
                    ih = hiota[:, 0:2 * D]
                    ih_rep = bass.AP(
                        ih.tensor, ih.offset,
                        [ih.ap[0], [0, nch], [1, 2 * D]])
                    prb = prtt[:, pl.ch0:pl.ch0 + nch].to_broadcast(
                        [P, nch, 2 * D])
                    nc.vector.tensor_tensor(
                        out=M[:, :].rearrange("p (a b) -> p a b", a=nch),
                        in0=ih_rep, in1=prb, op=OP.is_equal)
                    st["M"] = M
                    Sall = sallp.tile([P, nch * _SPAN], f16, tag="S")
                    st["Sall"] = Sall
                    ia = iot[:, 0:_SPAN]
                    iota_rep = bass.AP(
                        ia.tensor, ia.offset,
                        [ia.ap[0], [0, nch], [1, _SPAN]])
                    cdb = cdstt[:, pl.ch0:pl.ch0 + nch].to_broadcast(
                        [P, nch, _SPAN])
                    nc.vector.tensor_tensor(
                        out=Sall[:, :].rearrange("p (a b) -> p a b", a=nch),
                        in0=iota_rep, in1=cdb, op=OP.is_equal)
                    ps = psA.tile([2 * D, Lp], f32, tag="psA")
                    st["ps"] = ps
                    nc.vector.memset(ps[:, :], 0.0)
                    if pl.nch_old > 0:
                        nc.vector.tensor_tensor(
                            out=G[:, 0:pl.nch_old * 2 * D],
                            in0=G[:, 0:pl.nch_old * 2 * D],
                            in1=M[:, 0:pl.nch_old * 2 * D], op=OP.mult)
                        emit_aggs(pl, st, range(pl.nch_old))
                state[i] = st

            def emit_main(i):
                pl = plans[i]
                Lp, nch, nw2 = pl.Lp, pl.nch, pl.nw2
                st = state.pop(i)
                lhs = st["lhs"]
                if nch > 0:
                    G, M, ps = st["G"], st["M"], st["ps"]
                    if nch > pl.nch_old:
                        emit_gather(pl.calls, "fresh", G, pl.tb)
                        nc.vector.tensor_tensor(
                            out=G[:, pl.nch_old * 2 * D:nch * 2 * D],
                            in0=G[:, pl.nch_old * 2 * D:nch * 2 * D],
                            in1=M[:, pl.nch_old * 2 * D:nch * 2 * D],
                            op=OP.mult)
                        emit_aggs(pl, st, range(pl.nch_old, nch))
                    hsum = msgp.tile([64, Lp], f16, tag="hsum")
                    nc.vector.tensor_copy(out=hsum[:, :], in_=ps[0:D, :])
                    nc.vector.tensor_tensor(
                        out=lhs[0:64, :], in0=hsum[:, :],
                        in1=ps[D:2 * D, :], op=OP.add)
                else:
                    nc.vector.memset(lhs[0:64, :], 0.0)

                msg = msgp.tile([P, nw2 * D], f16, tag="msg")
                wcol = 0 if pl.sweep == "f" else D
                for w2 in range(nw2):
                    ps2 = psB.tile([P, D], f32, tag="ps2")
                    nc.tensor.matmul(
                        out=ps2[:, :],
                        lhsT=lhs[:, w2 * P:(w2 + 1) * P],
                        rhs=wt[:, wcol:wcol + D],
                        start=True, stop=True)
                    nc.vector.tensor_copy(
                        out=msg[:, w2 * D:(w2 + 1) * D], in_=ps2[:, :])
                nc.sync.dma_start(
                    out=agin[pl.cnt0:pl.cnt0 + Lp, :].rearrange(
                        "(j p) f -> p j f", p=P),
                    in_=msg[:, :].rearrange("p (j f) -> p j f", f=D))
                nc.gpsimd.collective_compute(
                    "AllGather", OP.bypass, replica_groups=rg,
                    ins=[agin[pl.cnt0:pl.cnt0 + Lp, :]],
                    outs=[T[pl.tb:pl.tb + C * pl.Lp, :]])

            def emit_ro_gatherA(stages, GA):
                for cl in ro["callsA"]:
                    if cl.fresh not in stages:  # .fresh holds the stage id
                        continue
                    n = cl.n
                    # call output goes at its slot range [base..base+n)
                    base = 0
                    for c2 in ro["callsA"]:
                        if c2 is cl:
                            break
                        base += c2.n
                    j0 = base // P
                    k = n // P
                    q = qctr[0] % NQ
                    qctr[0] += 1
                    nc.gpsimd.dma_gather(
                        out_ap=GA[:, j0 * 2 * D:(j0 + k) * 2 * D].rearrange(
                            "p (j f) -> p j f", f=2 * D),
                        in_ap=T[cl.base_pair * 2:
                                (cl.base_pair + cl.ext_pair) * 2,
                                :].rearrange("(q x) f -> q (x f)", x=2),
                        idxs_ap=idxAt[:, cl.icol0:cl.icol0 + n // 16],
                        num_idxs=n, num_idxs_reg=n, elem_size=2 * D,
                        single_packet=False, queue_num=q)

            # ---------------- level sweeps ----------------
            GA_ro = None
            if not skip_ro:
                GA_ro = ropool.tile([P, ro["kA"] * 2 * D], f16, tag="GA")
            if NP_ > 0:
                emit_prep(0)
            for i in range(NP_):
                if not skip_ro and i == NP_ - 1 and NP_ >= 2:
                    emit_ro_gatherA({0}, GA_ro)  # oldest: overlap lvl NP_-2
                emit_main(i)
                if i + 1 < NP_:
                    emit_prep(i + 1)
                if not skip_ro and i == NP_ - 1:
                    emit_ro_gatherA({1} if NP_ >= 2 else {0, 1}, GA_ro)

            # ---------------- readout ----------------
            if skip_ro:
                zo = smp.tile([B, 2 * D], f32, tag="outt")
                nc.vector.memset(zo[:, :], 0.0)
                nc.sync.dma_start(out=out_x[:, :], in_=zo[:, :])
                if dump_T:
                    nc.sync.dma_start(out=tdump_x[:, :], in_=T[:, :])
            else:
                emit_ro_gatherA({2}, GA_ro)
                # staging write: stg pair-row p*kA+j <- GA[p, j]
                nc.sync.dma_start(
                    out=stg_t[0:ro["A_tot"], :].rearrange(
                        "(p j) f -> p j f", j=ro["kA"]),
                    in_=GA_ro[:, :].rearrange("p (j f) -> p j f", f=2 * D))
                # phase C gather from staging
                R = ropool.tile([P, NRCH * 2 * D], f16, tag="R")
                q = qctr[0] % NQ
                qctr[0] += 1
                nc.gpsimd.dma_gather(
                    out_ap=R[:, :].rearrange("p (j f) -> p j f", f=2 * D),
                    in_ap=stg_t[:, :],
                    idxs_ap=idxAt[:, ro["icolC"]:
                                  ro["icolC"] + NRCH * P // 16],
                    num_idxs=NRCH * P, num_idxs_reg=NRCH * P,
                    elem_size=2 * D, single_packet=False, queue_num=q)
                # sum pool: psR[g, f] += S16[:, j]ᵀ @ R_par[:, j]
                S16 = sallp.tile([P, NRCH * 16], f16, tag="s16")
                i16t = io16[:, 0:16]
                i16_rep = bass.AP(
                    i16t.tensor, i16t.offset,
                    [i16t.ap[0], [0, NRCH], [1, 16]])
                rgb = rgidt[:, :].to_broadcast([P, NRCH, 16])
                nc.vector.tensor_tensor(
                    out=S16[:, :].rearrange("p (a b) -> p a b", a=NRCH),
                    in0=i16_rep, in1=rgb, op=OP.is_equal)
                ps_sum = psR.tile([B, D], f32)
                for j in range(NRCH):
                    par = ro["chunk_par"][j]
                    nc.tensor.matmul(
                        out=ps_sum[:, :],
                        lhsT=S16[:, j * 16:(j + 1) * 16],
                        rhs=R[:, j * 2 * D + par * D:j * 2 * D + par * D + D],
                        start=(j == 0), stop=(j == NRCH - 1))
                # max pool per graph: DVE tree over its chunks
                maxT = constp.tile([64, B], f32)
                for g, chs in enumerate(meta["graph_chunks"]):
                    def sl(j):
                        par = ro["chunk_par"][j]
                        return R[:, j * 2 * D + par * D:
                                 j * 2 * D + par * D + D]
                    mxf = smp.tile([P, D], f32, tag="mxf")
                    if len(chs) == 1:
                        nc.vector.tensor_copy(out=mxf[:, :], in_=sl(chs[0]))
                    else:
                        mx = smp.tile([P, D], f16, tag="mx")
                        nc.vector.tensor_tensor(
                            out=mx[:, :], in0=sl(chs[0]), in1=sl(chs[1]),
                            op=OP.max)
                        for j in chs[2:]:
                            nc.vector.tensor_tensor(
                                out=mx[:, :], in0=mx[:, :], in1=sl(j),
                                op=OP.max)
                        nc.vector.tensor_copy(out=mxf[:, :], in_=mx[:, :])
                    pst = psC.tile([64, P], f32, tag="psA")
                    nc.tensor.transpose(
                        out=pst[:, :], in_=mxf[:, :], identity=ident[:, :])
                    nc.vector.reduce_max(
                        out=maxT[:, g:g + 1], in_=pst[:, :], axis=AX.X)
                psmx = psC.tile([B, 64], f32, tag="psA")
                nc.tensor.transpose(
                    out=psmx[:, :], in_=maxT[:, :],
                    identity=ident[0:64, 0:64])
                pr = smp.tile([B, 2 * D], f32, tag="pr")
                nc.vector.tensor_copy(out=pr[:, 0:D], in_=psmx[:, :])
                nc.vector.tensor_copy(out=pr[:, D:2 * D], in_=ps_sum[:, :])
                nc.sync.dma_start(
                    out=prin[:, :].rearrange("(h g) f -> g h f", g=B),
                    in_=pr[:, :].rearrange("g (h f) -> g h f", h=2))
                nc.gpsimd.collective_compute(
                    "AllGather", OP.bypass, replica_groups=rg,
                    ins=[prin[:, :]], outs=[prout[:, :]])
                pr3 = prout[:, :].rearrange("(r gg) f -> gg r f", r=C)
                mx8 = smp.tile([B, C * D], f32, tag="mx8")
                sm8 = smp.tile([B, C * D], f32, tag="sm8")
                nc.sync.dma_start(
                    out=mx8[:, :].rearrange("g (r f) -> g r f", f=D),
                    in_=pr3[0:B])
                nc.sync.dma_start(
                    out=sm8[:, :].rearrange("g (r f) -> g r f", f=D),
                    in_=pr3[B:2 * B])
                outt = smp.tile([B, 2 * D], f32, tag="outt")
                for buf, op, o0 in ((mx8, OP.max, 0), (sm8, OP.add, D)):
                    nc.vector.tensor_tensor(
                        out=buf[:, 0:4 * D], in0=buf[:, 0:4 * D],
                        in1=buf[:, 4 * D:8 * D], op=op)
                    nc.vector.tensor_tensor(
                        out=buf[:, 0:2 * D], in0=buf[:, 0:2 * D],
                        in1=buf[:, 2 * D:4 * D], op=op)
                    nc.vector.tensor_tensor(
                        out=outt[:, o0:o0 + D], in0=buf[:, 0:D],
                        in1=buf[:, D:2 * D], op=op)
                nc.sync.dma_start(out=out_x[:, :], in_=outt[:, :])
                if dump_T:
                    nc.sync.dma_start(out=tdump_x[:, :], in_=T[:, :])
    nc.compile()
    return nc


def _in_maps(meta, arrays):
    maps = []
    for c in range(C):
        maps.append(dict(
            tab=arrays["tab"],
            wext=arrays["wext"],
            iota=arrays["iota512"],
            iota16=arrays["iota16"],
            idxA=np.ascontiguousarray(arrays["idxA"][c]),
            cdst=np.ascontiguousarray(arrays["cdst"][c]),
            prt=np.ascontiguousarray(arrays["prt"][c]),
            cnts=np.ascontiguousarray(arrays["cnts"][c]),
            rgid=np.ascontiguousarray(arrays["rgid"][c]),
        ))
    return maps


_LAST_RESULTS = None  # stash for test harness (exec time, trace)


def kernel(**inputs):
    global _LAST_RESULTS
    import os
    meta, arrays = _preprocess(**inputs)
    nc = _build(meta)
    from concourse.bass_utils import run_bass_kernel_spmd
    res = run_bass_kernel_spmd(nc, _in_maps(meta, arrays),
                               core_ids=list(range(C)),
                               trace=bool(os.environ.get("KERNEL_TRACE")))
    _LAST_RESULTS = res
    return np.asarray(res.results[0]["out"])


# revision 51
# speedup vs baseline: 1.7741x; 1.1787x over previous
"""DAG-GNN level-sweep kernel for Trainium2 (8 NeuronCores, Bass/Tile), v2.

Design vs v1:
  - State table T in DRAM is fp16 (tolerance 2e-2 >> fp16's ~3e-4 here);
    gathers fetch PAIRS of consecutive node rows (256B) so the int16
    dma_gather index window covers 65536 node rows, and each level's
    sources fit in a handful of windows.
  - ONE-hop gather per level (no staging round-trip, no permutation
    gather): slots sorted by (window, parity, dst col); chunks are
    window- and parity-pure; the per-chunk selection matmul picks the
    even or odd half of each gathered pair.
  - Old/fresh split: gathers whose sources predate the previous level's
    AllGather are emitted one level early so they overlap the previous
    level's compute + collective; only the small "fresh" gather (sources
    in the immediately preceding level block) sits on the critical path.
  - AllGather moves fp16 (half the bytes of v1).
  - Readout keeps a two-hop (phase A windows -> staging -> graph/parity
    grouped chunks) but phase A overlaps the last levels.
"""

import sys

if "/opt/trn_rl_repo" not in sys.path:
    sys.path.insert(0, "/opt/trn_rl_repo")

import numpy as np

# structural constants of the nn.Module (match reference)
B = 16   # graphs per batch
LF = 12  # forward topological levels
LB = 12  # backward topological levels
D = 64   # hidden dim
C = 8    # NeuronCores
P = 128  # SBUF partitions

# T table header rows (node-row space; pairs are rows (2q, 2q+1))
_H0 = 1        # 9 rows of initial-encoding table, indexed by 3*nt + nip
_MI0 = 10      # fp16-min row, even parity
_MI1 = 11      # fp16-min row, odd parity
_HDR = 12

_WPAIR = 32768   # pair rows addressable by one int16 gather window
_F16MIN = -65504.0


def _ceil(a, b):
    return -(-a // b)


_SPAN = 512   # bulk selection-matrix width per entry (max matmul width)
_EMAX = 2048  # entry col range (fp16-exact integer compares)


class _Lvl:
    __slots__ = (
        "sweep", "l", "Lp", "tb", "nw2", "cnt0",
        "calls", "nch", "ch0", "eh0", "nch_old", "entries",
    )


class _Call:
    """One dma_gather call: window [base_pair, base_pair+ext_pair) of T."""
    __slots__ = ("base_pair", "ext_pair", "icol0", "n", "fresh")

    def __init__(self, base_pair, ext_pair, icol0, n, fresh):
        self.base_pair = base_pair
        self.ext_pair = ext_pair
        self.icol0 = icol0
        self.n = n
        self.fresh = fresh


def _wrap_idx(a):
    """[C, n] int -> [C, 128, n/16] int16 (16-partition wrap, 8 replicas)."""
    Csz, n = a.shape
    w = a.reshape(Csz, n // 16, 16).transpose(0, 2, 1).astype(np.int16)
    return np.ascontiguousarray(np.tile(w, (1, 8, 1)))


def _preprocess(node_type, num_inverted_predecessors, edge_index,
                forward_level, backward_level, batch,
                W_enc, b_enc, W_f, b_f, W_b, b_b):
    N = int(node_type.shape[0])
    nt = np.asarray(node_type).astype(np.int64)
    nip = np.asarray(num_inverted_predecessors).astype(np.int64)
    fl = np.asarray(forward_level).astype(np.int64)
    bl = np.asarray(backward_level).astype(np.int64)
    bt = np.asarray(batch).astype(np.int64)
    src = np.asarray(edge_index[0]).astype(np.int64)
    dst = np.asarray(edge_index[1]).astype(np.int64)
    code = nt * 3 + nip  # in [0, 9)

    # ---------------- node positions in T ----------------
    posf = np.full(N, -1, np.int64)
    posb = np.full(N, -1, np.int64)
    coref = np.zeros(N, np.int64)
    rankf = np.zeros(N, np.int64)
    coreb = np.zeros(N, np.int64)
    rankb = np.zeros(N, np.int64)

    tbase = _HDR
    lvl_meta = []
    for sweep, lv, pos, core, rank, nl in (
        ("f", fl, posf, coref, rankf, LF),
        ("b", bl, posb, coreb, rankb, LB),
    ):
        for l in range(1, nl):
            idx = np.flatnonzero(lv == l)
            n_l = idx.size
            if n_l == 0:
                lvl_meta.append((sweep, l, 0, tbase))
                continue
            Lp = _ceil(_ceil(n_l, C), P) * P
            c = np.arange(n_l) % C
            r = np.arange(n_l) // C
            pos[idx] = tbase + c * Lp + r
            core[idx] = c
            rank[idx] = r
            lvl_meta.append((sweep, l, Lp, tbase))
            tbase += C * Lp
    T_ROWS = tbase

    # ---------------- per-level plans ----------------
    plans = []
    cdst_blocks = []
    prt_blocks = []
    cnts_blocks = []
    idx_cols = []      # list of [C, 128, k] int16 blocks
    icol0 = 0
    cnt0 = 0
    ch0 = 0
    eh0 = 0
    fl_dst = fl[dst]
    bl_src = bl[src]
    prev_block = None  # (tb, rows) of previous plan's T block
    for (sweep, l, Lp, tb) in lvl_meta:
        if Lp == 0:
            continue
        pl = _Lvl()
        pl.sweep, pl.l, pl.Lp, pl.tb = sweep, l, Lp, tb
        pl.nw2 = Lp // P
        pl.cnt0 = cnt0
        if sweep == "f":
            em = fl_dst == l
            un = dst[em]
            dn = src[em]
            ucore = coref[un]
            urank = rankf[un]
            gat = (fl[dn] >= 1) & (fl[dn] < l)
            gpos_all = posf[dn]
        else:
            em = bl_src == l
            un = src[em]
            dn = dst[em]
            ucore = coreb[un]
            urank = rankb[un]
            upd_b = (bl[dn] >= 1) & (bl[dn] < l)
            upd_f = (~upd_b) & (fl[dn] >= 1)
            gat = upd_b | upd_f
            gpos_all = np.where(upd_b, posb[dn], posf[dn])

        # counts: initial-valued sources by code, plus total indegree (bias)
        cnt = np.zeros((C, Lp, 10), np.float32)
        i0 = ~gat
        np.add.at(cnt, (ucore[i0], urank[i0], code[dn[i0]]), 1.0)
        np.add.at(cnt, (ucore, urank, 9), 1.0)
        cnts_blocks.append(np.ascontiguousarray(
            cnt.transpose(0, 2, 1)).astype(np.float16))
        cnt0 += Lp

        gc = ucore[gat]
        gr = urank[gat]        # dst col (rank == col)
        gp = gpos_all[gat]     # source node row in T
        if gp.size == 0:
            pl.calls, pl.nch, pl.ch0, pl.eh0 = [], 0, ch0, eh0
            pl.nch_old, pl.entries = 0, []
            plans.append(pl)
            prev_block = (tb, C * Lp)
            continue

        # window id: old region [0, fs) in _WPAIR-pair windows; fresh = prev
        fs = prev_block[0]
        fresh_rows = prev_block[1]
        n_oldwin = _ceil(fs // 2, _WPAIR) if fs > 0 else 0
        wid = np.where(gp < fs, (gp // 2) // _WPAIR, n_oldwin)
        par = gp % 2
        poff = np.where(gp < fs, (gp // 2) % _WPAIR, gp // 2 - fs // 2)

        # group sizes per window (parity folded into the mask): uniform
        gsz = np.zeros(n_oldwin + 1, np.int64)
        for w in range(n_oldwin + 1):
            mx = 0
            for c in range(C):
                mx = max(mx, int(((gc == c) & (wid == w)).sum()))
            gsz[w] = _ceil(mx, P) * P if mx else 0
        S_tot = int(gsz.sum())
        nch = S_tot // P

        # slot arrays [C, S_tot]
        off_sl = np.zeros((C, S_tot), np.int64)   # pair offset in window
        col_sl = np.full((C, S_tot), -1, np.int64)  # dst col (-1 padding)
        par_sl = np.zeros((C, S_tot), np.int64)
        gstart = np.concatenate([[0], np.cumsum(gsz)])[:-1]
        for c in range(C):
            m_c = gc == c
            order = np.lexsort((gr[m_c], wid[m_c]))
            wo = wid[m_c][order]
            oo = poff[m_c][order]
            ro = gr[m_c][order]
            qo = par[m_c][order]
            cnts_g = np.bincount(wo, minlength=n_oldwin + 1)
            st = np.concatenate([[0], np.cumsum(cnts_g)])[:-1]
            seq = np.arange(wo.size) - st[wo]
            slot = gstart[wo] + seq
            off_sl[c, slot] = oo
            col_sl[c, slot] = ro
            par_sl[c, slot] = qo

        # gather calls: one per window with slots
        calls = []
        nch_old = 0
        for w in range(n_oldwin + 1):
            n_g = int(gsz[w])
            if n_g == 0:
                continue
            if w < n_oldwin:
                base = w * _WPAIR
                ext = min(_WPAIR, fs // 2 - base)
                fresh = False
                nch_old += n_g // P
            else:
                base = fs // 2
                ext = fresh_rows // 2
                fresh = True
            a = int(gstart[w])
            blk = off_sl[:, a:a + n_g]
            idx_cols.append(_wrap_idx(blk))
            calls.append(_Call(base, ext, icol0, n_g, fresh))
            icol0 += n_g // 16
        pl.calls = calls
        pl.nch_old = nch_old

        # per-chunk entries (col range <= _EMAX each) with exact-span pieces
        entries = []   # (j, c0_abs, pieces, ecol)
        ecd_cols = []  # per-entry cdst column [C, P]
        for j in range(nch):
            v = col_sl[:, j * P:(j + 1) * P]   # [C, P]
            vv = v[v >= 0]
            if vv.size == 0:
                continue
            lo, hi = int(vv.min()), int(vv.max())
            for b in range(lo, hi + 1, _EMAX):
                sel = (v >= b) & (v < b + _EMAX)
                if not sel.any():
                    continue
                emax = int(v[sel].max()) - b
                cdv = np.where(sel, v - b, -1).astype(np.float16)
                pc = []
                k = 0
                span = emax + 1
                while span > 0:
                    w_k = min(_SPAN, span)
                    pc.append((k, w_k))
                    span -= w_k
                    k += 1
                entries.append((j, b, pc, len(ecd_cols)))
                ecd_cols.append(cdv)
        nE = len(ecd_cols)
        cdst_blocks.append(np.ascontiguousarray(
            np.stack(ecd_cols, axis=2)) if nE
            else np.full((C, P, 1), -1.0, np.float16))
        prt_blocks.append(np.ascontiguousarray(
            par_sl.reshape(C, nch, P).transpose(0, 2, 1)).astype(np.float16))
        pl.nch, pl.ch0, pl.eh0 = nch, ch0, eh0
        pl.entries = entries
        ch0 += nch
        eh0 += max(nE, 1)
        plans.append(pl)
        prev_block = (tb, C * Lp)

    TOTCH = max(1, ch0)
    TOTE = max(1, eh0)
    CNT_TOT = cnt0
    cdst_all = (np.concatenate(cdst_blocks, axis=2) if cdst_blocks
                else np.full((C, P, 1), -1.0, np.float16))
    prt_all = (np.concatenate(prt_blocks, axis=2) if prt_blocks
               else np.zeros((C, P, 1), np.float16))
    if cdst_all.shape[2] < TOTE:
        pad = TOTE - cdst_all.shape[2]
        cdst_all = np.concatenate(
            [cdst_all, np.full((C, P, pad), -1.0, np.float16)], axis=2)
    if prt_all.shape[2] < TOTCH:
        pad = TOTCH - prt_all.shape[2]
        prt_all = np.concatenate(
            [prt_all, np.zeros((C, P, pad), np.float16)], axis=2)
    cnts_all = np.concatenate(cnts_blocks, axis=2)

    # ---------------- readout layout ----------------
    last_tb = plans[-1].tb
    last_rows = C * plans[-1].Lp
    mid_tb = plans[-2].tb
    onodes = np.flatnonzero(nt == 1)
    og = bt[onodes]
    fpos = np.where(bl[onodes] >= 1, posb[onodes],
                    np.where(fl[onodes] >= 1, posf[onodes],
                             _H0 + code[onodes]))
    oc = np.arange(onodes.size) % C  # core assignment

    # --- phase A: slots grouped per window; stage order = arrival order
    # stage 0: rows < mid_tb (oldest); 1: [mid_tb, last_tb); 2: last block
    n_oldwinA = _ceil(mid_tb // 2, _WPAIR)
    widA = np.where(fpos < mid_tb, (fpos // 2) // _WPAIR,
                    np.where(fpos < last_tb, n_oldwinA, n_oldwinA + 1))
    basesA = [w * _WPAIR for w in range(n_oldwinA)] + \
        [mid_tb // 2, last_tb // 2]
    extsA = [min(_WPAIR, mid_tb // 2 - w * _WPAIR)
             for w in range(n_oldwinA)] + \
        [(last_tb - mid_tb) // 2, last_rows // 2]
    stageA = [0] * n_oldwinA + [1, 2]
    nwinA = n_oldwinA + 2
    # per (window) sizes uniform across cores; slot 0 of window 0 reserved
    # for the fp16-min pair (pair row 5) used as padding target.
    poffA = fpos // 2 - np.asarray(basesA, np.int64)[widA]
    gszA = np.zeros(nwinA, np.int64)
    for w in range(nwinA):
        mx = 0
        for c in range(C):
            mx = max(mx, int(((oc == c) & (widA == w)).sum()))
        mx += (1 if w == 0 else 0)  # reserved pad slot
        gszA[w] = _ceil(mx, P) * P if mx else 0
    A_tot = int(gszA.sum())
    kA = A_tot // P
    gstartA = np.concatenate([[0], np.cumsum(gszA)])[:-1]
    offA = np.zeros((C, A_tot), np.int64)
    offA[:, 0] = _MI0 // 2  # pad pair (both halves fp16-min)
    # staged row of phase-A slot q (GA[p, j] -> stg row p*kA+j)
    stg_row = (np.arange(A_tot) % P) * kA + np.arange(A_tot) // P
    posA = np.zeros((C, onodes.size), np.int64)  # node -> phase-A slot
    for c in range(C):
        m_c = oc == c
        order = np.argsort(widA[m_c], kind="stable")
        wo = widA[m_c][order]
        oo = poffA[m_c][order]
        cnts_g = np.bincount(wo, minlength=nwinA)
        st = np.concatenate([[0], np.cumsum(cnts_g)])[:-1]
        seq = np.arange(wo.size) - st[wo]
        slot = gstartA[wo] + seq + (wo == 0)  # skip reserved slot 0
        offA[c, slot] = oo
        posA[c, np.flatnonzero(m_c)[order]] = slot
    ro_callsA = []
    icolA_blocks = []
    for w in range(nwinA):
        n_g = int(gszA[w])
        if n_g == 0:
            continue
        a = int(gstartA[w])
        icolA_blocks.append(offA[:, a:a + n_g])
        ro_callsA.append(_Call(basesA[w], extsA[w], icol0, n_g, stageA[w]))
        icol0 += n_g // 16
    for blk in icolA_blocks:
        idx_cols.append(_wrap_idx(blk))

    # --- phase C: final chunks grouped by (graph, parity)
    parR = fpos % 2
    kg2 = np.zeros((B, 2), np.int64)
    for g in range(B):
        for q in range(2):
            mx = 0
            for c in range(C):
                mx = max(mx, int(((oc == c) & (og == g) &
                                  (parR == q)).sum()))
            kg2[g, q] = _ceil(max(mx, 1), P) * P
    R_tot = int(kg2.sum())
    NRCH = R_tot // P
    gstartR = np.concatenate([[0], np.cumsum(kg2.reshape(-1))])[:-1]
    gstartR = gstartR.reshape(B, 2)
    idxC = np.zeros((C, R_tot), np.int64)
    idxC[:, :] = int(stg_row[0])  # padding -> staged fp16-min pair
    rgid = np.full((C, R_tot), -1.0, np.float32)
    chunk_parR = np.zeros(NRCH, np.int64)
    graph_chunks = []  # per graph: list of chunk ids
    for g in range(B):
        chs = []
        for q in range(2):
            a, n_g = int(gstartR[g, q]), int(kg2[g, q])
            chunk_parR[a // P:(a + n_g) // P] = q
            chs.extend(range(a // P, (a + n_g) // P))
        graph_chunks.append(chs)
    for c in range(C):
        m_c = oc == c
        order = np.lexsort((parR[m_c], og[m_c]))
        go = og[m_c][order]
        qo = parR[m_c][order]
        so = posA[c, m_c][order]
        grp = go * 2 + qo
        cnts_g = np.bincount(grp, minlength=B * 2)
        st = np.concatenate([[0], np.cumsum(cnts_g)])[:-1]
        seq = np.arange(go.size) - st[grp]
        slot = gstartR[go, qo] + seq
        idxC[c, slot] = stg_row[so]
        rgid[c, slot] = go
    icolC = icol0
    idx_cols.append(_wrap_idx(idxC))
    icol0 += R_tot // 16
    rgid_arr = np.ascontiguousarray(
        rgid.reshape(C, NRCH, P).transpose(0, 2, 1)).astype(np.float16)

    # ---------------- weight-derived constants ----------------
    W_enc = np.asarray(W_enc, np.float32)
    b_enc = np.asarray(b_enc, np.float32)
    W_f = np.asarray(W_f, np.float32)
    b_f = np.asarray(b_f, np.float32)
    W_b = np.asarray(W_b, np.float32)
    b_b = np.asarray(b_b, np.float32)
    h0_tab = np.zeros((9, D), np.float32)
    for cc in range(9):
        h0_tab[cc] = (cc // 3) * W_enc[0] + (cc % 3) * W_enc[1] + b_enc
    tab = np.zeros((_HDR, D), np.float32)
    tab[_H0:_H0 + 9] = h0_tab
    tab[_MI0] = _F16MIN
    tab[_MI1] = _F16MIN
    wf_ext = np.concatenate([W_f, h0_tab @ W_f, b_f[None, :]], axis=0)
    wb_ext = np.concatenate([W_b, h0_tab @ W_b, b_b[None, :]], axis=0)
    wext = np.ascontiguousarray(np.concatenate([wf_ext, wb_ext], axis=1))

    iota512 = np.ascontiguousarray(
        np.tile(np.arange(512, dtype=np.float16), (P, 1)))
    iota16 = np.ascontiguousarray(
        np.tile(np.arange(16, dtype=np.float16), (P, 1)))

    idxA_all = np.concatenate(idx_cols, axis=2)
    meta = dict(
        plans=plans, graph_chunks=graph_chunks,
        T_ROWS=T_ROWS, AG_ROWS=max(1, CNT_TOT), TOTCH=TOTCH, TOTE=TOTE,
        CNT_TOT=max(1, CNT_TOT), NRCH=NRCH, AW=idxA_all.shape[2],
        ro=dict(callsA=ro_callsA, kA=kA, A_tot=A_tot, icolC=icolC,
                chunk_par=chunk_parR.tolist()),
    )
    arrays = dict(
        tab=tab.astype(np.float16), wext=wext.astype(np.float16),
        iota512=iota512, iota16=iota16, idxA=idxA_all,
        cdst=cdst_all, prt=prt_all, cnts=cnts_all, rgid=rgid_arr,
    )
    return meta, arrays


# ---------------------------------------------------------------------------
# pure-numpy execution of the plan (host self-check / debugging)
# ---------------------------------------------------------------------------

def _sim_gather(T16, arrays, c, calls, n_slots):
    """numpy mirror of the device one-hop pair gather -> [n_slots, 2, D]."""
    idxA = arrays["idxA"][c][0:16, :]
    out = np.zeros((n_slots, 2, D), np.float32)
    Tp = T16.reshape(-1, 2, D)
    base = 0
    for cl in calls:
        cols = slice(cl.icol0, cl.icol0 + cl.n // 16)
        off = idxA[:, cols].T.reshape(-1)[:cl.n].astype(np.int64)
        assert off.max(initial=0) < cl.ext_pair
        out[base:base + cl.n] = Tp[cl.base_pair + off]
        base += cl.n
    assert base == n_slots
    return out


def _simulate_plan(meta, arrays, fp16=True, return_T=False):
    def rnd(x):
        return (np.clip(x, _F16MIN, -_F16MIN).astype(np.float16)
                .astype(np.float32) if fp16 else x)

    T = np.zeros((meta["T_ROWS"], D), np.float32)
    T[0:_HDR] = arrays["tab"].astype(np.float32)
    wext = arrays["wext"].astype(np.float32)
    for pl in meta["plans"]:
        wmat = wext[:, 0:D] if pl.sweep == "f" else wext[:, D:2 * D]
        blocks = []
        for c in range(C):
            lhs = np.zeros((74, pl.Lp), np.float32)
            lhs[64:74] = arrays["cnts"][c, :, pl.cnt0:pl.cnt0 + pl.Lp]
            if pl.nch > 0:
                prv = arrays["prt"][c][:, pl.ch0:pl.ch0 + pl.nch].astype(
                    np.float32)
                Gp = _sim_gather(T, arrays, c, pl.calls, pl.nch * P)
                # G[p, j, 2*D] = slot j*P+p (full pair), parity-masked
                G = Gp.reshape(pl.nch, P, 2 * D).transpose(1, 0, 2).copy()
                half = (np.arange(2 * D) // D)[None, None, :]
                G *= (prv[:, :, None] == half)
                ps = np.zeros((2 * D, pl.Lp), np.float32)
                for (j, b, pieces, ecol) in pl.entries:
                    cdv = arrays["cdst"][c][:, pl.eh0 + ecol].astype(
                        np.float32)
                    for (k, w_k) in pieces:
                        a = b + k * _SPAN
                        S = (cdv[:, None] ==
                             (np.arange(w_k) + k * _SPAN)[None, :])
                        ps[:, a:a + w_k] += G[:, j, :].T @ S.astype(
                            np.float32)
                lhs[0:64] = rnd(ps[0:D] + ps[D:2 * D])
            blocks.append(rnd(lhs.T @ wmat))  # [Lp, D]
        T[pl.tb:pl.tb + C * pl.Lp] = np.concatenate(blocks, axis=0)
    # readout
    ro = meta["ro"]
    NRCH = meta["NRCH"]
    maxp = np.full((B, D), _F16MIN, np.float32)
    sump = np.zeros((B, D), np.float32)
    for c in range(C):
        GA = _sim_gather(T, arrays, c, ro["callsA"], ro["A_tot"])
        # staged row of slot q = (q % P) * kA + q // P
        stg = np.zeros((ro["A_tot"], 2, D), np.float32)
        q = np.arange(ro["A_tot"])
        stg[(q % P) * ro["kA"] + q // P] = GA
        idxC = arrays["idxA"][c][0:16,
                                 ro["icolC"]:ro["icolC"] + NRCH * P // 16]
        off = idxC.T.reshape(-1).astype(np.int64)
        R_lin = stg[off]  # [NRCH*P, 2, D]
        R = R_lin.reshape(NRCH, P, 2, D)
        gid = arrays["rgid"][c]  # [P, NRCH]
        for g, chs in enumerate(meta["graph_chunks"]):
            for j in chs:
                sl = R[j, :, ro["chunk_par"][j], :]  # [P, D]
                maxp[g] = np.maximum(maxp[g], sl.max(axis=0))
                msk = (gid[:, j] == g).astype(np.float32)
                sump[g] += msk @ sl
    out = np.concatenate([maxp, sump], axis=1)
    return (out, T) if return_T else out


# ---------------------------------------------------------------------------
# Bass program
# ---------------------------------------------------------------------------

def _build(meta):
    import os

    import concourse.bass as bass
    import concourse.mybir as mybir
    from concourse import bacc, tile
    from concourse.masks import make_identity

    f32 = mybir.dt.float32
    f16 = mybir.dt.float16
    i16 = mybir.dt.int16
    AX = mybir.AxisListType
    OP = mybir.AluOpType

    TOTCH, CNT_TOT, NRCH = meta["TOTCH"], meta["CNT_TOT"], meta["NRCH"]
    TOTE = meta["TOTE"]
    AW = meta["AW"]
    plans = meta["plans"]
    NQ = 4  # SWDGE queues, round-robin over gather calls

    nc = bacc.Bacc(None, num_devices=C, num_swdge_queues=NQ)
    tab_x = nc.dram_tensor("tab", [_HDR, D], f16, kind="ExternalInput")
    wext_x = nc.dram_tensor("wext", [74, 2 * D], f16, kind="ExternalInput")
    iota_x = nc.dram_tensor("iota", [P, 512], f16, kind="ExternalInput")
    io16_x = nc.dram_tensor("iota16", [P, 16], f16, kind="ExternalInput")
    idxA_x = nc.dram_tensor("idxA", [P, AW], i16, kind="ExternalInput")
    cdst_x = nc.dram_tensor("cdst", [P, TOTE], f16, kind="ExternalInput")
    prt_x = nc.dram_tensor("prt", [P, TOTCH], f16, kind="ExternalInput")
    cnts_x = nc.dram_tensor("cnts", [10, CNT_TOT], f16, kind="ExternalInput")
    rgid_x = nc.dram_tensor("rgid", [P, NRCH], f16, kind="ExternalInput")
    out_x = nc.dram_tensor("out", [B, 2 * D], f32, kind="ExternalOutput")

    T = nc.dram_tensor("T", [meta["T_ROWS"], D], f16, addr_space="Shared")
    dump_T = bool(os.environ.get("KERNEL_DUMP_T"))
    tdump_x = (nc.dram_tensor("tdump", [meta["T_ROWS"], D], f16,
                              kind="ExternalOutput") if dump_T else None)
    agin = nc.dram_tensor("agin", [meta["AG_ROWS"], D], f16)
    ro = meta["ro"]
    stg_t = nc.dram_tensor("stg", [max(ro["A_tot"], P), 2 * D], f16)
    prin = nc.dram_tensor("prin", [2 * B, D], f32)
    prout = nc.dram_tensor("prout", [C * 2 * B, D], f32, addr_space="Shared")

    rg = [list(range(C))]
    qctr = [0]

    max_lv = int(os.environ.get("KERNEL_MAX_LEVELS", "99"))
    skip_ro = bool(os.environ.get("KERNEL_SKIP_READOUT"))
    plans = plans[:max_lv]
    NP_ = len(plans)

    with tile.TileContext(nc) as tc:
        with (
            tc.tile_pool(name="const", bufs=1) as constp,
            tc.tile_pool(name="g", bufs=2) as gpool,
            tc.tile_pool(name="ro", bufs=1) as ropool,
            tc.tile_pool(name="sall", bufs=2) as sallp,
            tc.tile_pool(name="sx", bufs=4) as sxp,
            tc.tile_pool(name="lhs", bufs=2) as lhsp,
            tc.tile_pool(name="msg", bufs=2) as msgp,
            tc.tile_pool(name="small", bufs=1) as smp,
            tc.tile_pool(name="psA", bufs=1, space="PSUM") as psA,
            tc.tile_pool(name="psB", bufs=1, space="PSUM") as psB,
        ):
            psC = psA  # readout PSUM tiles reuse the (dead) psA slot
            psR = psA
            # ---------------- setup ----------------
            stg0t = constp.tile([_HDR, D], f16)
            nc.sync.dma_start(out=stg0t[:, :], in_=tab_x[:, :])
            nc.sync.dma_start(out=T[0:_HDR, :], in_=stg0t[:, :])
            wt = constp.tile([74, 2 * D], f16)
            nc.sync.dma_start(out=wt[:, :], in_=wext_x[:, :])
            iot = constp.tile([P, 512], f16)
            nc.sync.dma_start(out=iot[:, :], in_=iota_x[:, :])
            io16 = constp.tile([P, 16], f16)
            nc.sync.dma_start(out=io16[:, :], in_=io16_x[:, :])
            cdstt = constp.tile([P, TOTE], f16)
            nc.sync.dma_start(out=cdstt[:, :], in_=cdst_x[:, :])
            cdst32 = constp.tile([P, TOTE], f32)
            nc.scalar.activation(
                cdst32[:, :], cdstt[:, :],
                mybir.ActivationFunctionType.Copy)
            prtt = constp.tile([P, TOTCH], f16)
            nc.sync.dma_start(out=prtt[:, :], in_=prt_x[:, :])
            hiota = constp.tile([P, 2 * D], f16)
            nc.vector.tensor_scalar(
                hiota[:, :], iot[:, 0:2 * D], float(D), None, OP.is_ge)
            idxAt = constp.tile([P, AW], i16)
            nc.sync.dma_start(out=idxAt[:, :], in_=idxA_x[:, :])
            rgidt = constp.tile([P, NRCH], f16)
            nc.sync.dma_start(out=rgidt[:, :], in_=rgid_x[:, :])
            ident = constp.tile([P, P], f32)
            make_identity(nc, ident[:, :])

            def emit_gather(calls, which, G, frontier):
                """Emit dma_gather calls; which selects fresh/old/all."""
                base = 0
                for cl in calls:
                    n = cl.n
                    j0 = base // P
                    k = n // P
                    base += n
                    if which == "fresh" and not cl.fresh:
                        continue
                    if which == "old" and cl.fresh:
                        continue
                    wrows = min(cl.ext_pair, frontier // 2 - cl.base_pair)
                    q = qctr[0] % NQ
                    qctr[0] += 1
                    nc.gpsimd.dma_gather(
                        out_ap=G[:, j0 * 2 * D:(j0 + k) * 2 * D].rearrange(
                            "p (j f) -> p j f", f=2 * D),
                        in_ap=T[cl.base_pair * 2:
                                (cl.base_pair + wrows) * 2, :].rearrange(
                            "(q x) f -> q (x f)", x=2),
                        idxs_ap=idxAt[:, cl.icol0:cl.icol0 + n // 16],
                        num_idxs=n, num_idxs_reg=n, elem_size=2 * D,
                        single_packet=False, queue_num=q)

            state = {}

            def emit_aggs(pl, st, old):
                G, ps, Sall = st["G"], st["ps"], st["Sall"]
                for (j, b, pieces, ecol) in pl.entries:
                    if (j < pl.nch_old) != old:
                        continue
                    for (k, w_k) in pieces:
                        a = b + k * _SPAN
                        if k == 0:
                            S_ap = Sall[:, ecol * _SPAN:ecol * _SPAN + w_k]
                        else:
                            S2 = sxp.tile([P, _SPAN], f16, tag="S2")
                            nc.vector.tensor_scalar(
                                S2[:, 0:w_k], iot[:, 0:w_k],
                                cdst32[:, pl.eh0 + ecol:pl.eh0 + ecol + 1],
                                float(-k * _SPAN), OP.subtract, OP.is_equal)
                            S_ap = S2[:, 0:w_k]
                        nc.tensor.matmul(
                            out=ps[:, a:a + w_k],
                            lhsT=G[:, j * 2 * D:(j + 1) * 2 * D],
                            rhs=S_ap,
                            start=False, stop=False, skip_group_check=True)

            def emit_prep(i):
                """S build (DVE), counts DMA, old gathers, old aggs."""
                pl = plans[i]
                Lp, nch = pl.Lp, pl.nch
                lhs = lhsp.tile([74, Lp], f16, tag="lhs")
                nc.sync.dma_start(
                    out=lhs[64:74, :],
                    in_=cnts_x[:, pl.cnt0:pl.cnt0 + Lp])
                st = dict(lhs=lhs)
                if nch > 0:
                    G = gpool.tile([P, nch * 2 * D], f16, tag="G")
                    st["G"] = G
                    emit_gather(pl.calls, "old", G, meta["T_ROWS"] * 2)
                    # parity masks for all chunks + selection matrices
                    M = sallp.tile([P, nch * 2 * D], f16, tag="M")
                    ih = hiota[:, 0:2 * D]
                    ih_rep = bass.AP(
                        ih.tensor, ih.offset,
                        [ih.ap[0], [0, nch], [1, 2 * D]])
                    prb = prtt[:, pl.ch0:pl.ch0 + nch].to_broadcast(
                        [P, nch, 2 * D])
                    nc.vector.tensor_tensor(
                        out=M[:, :].rearrange("p (a b) -> p a b", a=nch),
                        in0=ih_rep, in1=prb, op=OP.is_equal)
                    st["M"] = M
                    nE = max(len(pl.entries), 1)
                    Sall = sallp.tile([P, nE * _SPAN], f16, tag="S")
                    st["Sall"] = Sall
                    ia = iot[:, 0:_SPAN]
                    iota_rep = bass.AP(
                        ia.tensor, ia.offset,
                        [ia.ap[0], [0, nE], [1, _SPAN]])
                    cdb = cdstt[:, pl.eh0:pl.eh0 + nE].to_broadcast(
                        [P, nE, _SPAN])
                    nc.vector.tensor_tensor(
                        out=Sall[:, :].rearrange("p (a b) -> p a b", a=nE),
                        in0=iota_rep, in1=cdb, op=OP.is_equal)
                    ps = psA.tile([2 * D, Lp], f32, tag="psA")
                    st["ps"] = ps
                    nc.vector.memset(ps[:, :], 0.0)
                    if pl.nch_old > 0:
                        nc.vector.tensor_tensor(
                            out=G[:, 0:pl.nch_old * 2 * D],
                            in0=G[:, 0:pl.nch_old * 2 * D],
                            in1=M[:, 0:pl.nch_old * 2 * D], op=OP.mult)
                        emit_aggs(pl, st, True)
                state[i] = st

            def emit_main(i):
                pl = plans[i]
                Lp, nch, nw2 = pl.Lp, pl.nch, pl.nw2
                st = state.pop(i)
                lhs = st["lhs"]
                if nch > 0:
                    G, M, ps = st["G"], st["M"], st["ps"]
                    if nch > pl.nch_old:
                        emit_gather(pl.calls, "fresh", G, pl.tb)
                        nc.vector.tensor_tensor(
                            out=G[:, pl.nch_old * 2 * D:nch * 2 * D],
                            in0=G[:, pl.nch_old * 2 * D:nch * 2 * D],
                            in1=M[:, pl.nch_old * 2 * D:nch * 2 * D],
                            op=OP.mult)
                        emit_aggs(pl, st, False)
                    hsum = msgp.tile([64, Lp], f16, tag="hsum")
                    nc.scalar.activation(
                        hsum[:, :], ps[0:D, :],
                        mybir.ActivationFunctionType.Copy)
                    nc.vector.tensor_tensor(
                        out=lhs[0:64, :], in0=hsum[:, :],
                        in1=ps[D:2 * D, :], op=OP.add)
                else:
                    nc.vector.memset(lhs[0:64, :], 0.0)

                msg = msgp.tile([P, nw2 * D], f16, tag="msg")
                wcol = 0 if pl.sweep == "f" else D
                psW = psB.tile([P, nw2 * D], f32, tag="psW")
                for w2 in range(nw2):
                    nc.tensor.matmul(
                        out=psW[:, w2 * D:(w2 + 1) * D],
                        lhsT=lhs[:, w2 * P:(w2 + 1) * P],
                        rhs=wt[:, wcol:wcol + D],
                        start=True, stop=True)
                nc.scalar.activation(
                    msg[:, :], psW[:, :], mybir.ActivationFunctionType.Copy)
                nc.sync.dma_start(
                    out=agin[pl.cnt0:pl.cnt0 + Lp, :].rearrange(
                        "(j p) f -> p j f", p=P),
                    in_=msg[:, :].rearrange("p (j f) -> p j f", f=D))
                nc.gpsimd.collective_compute(
                    "AllGather", OP.bypass, replica_groups=rg,
                    ins=[agin[pl.cnt0:pl.cnt0 + Lp, :]],
                    outs=[T[pl.tb:pl.tb + C * pl.Lp, :]])

            def emit_ro_gatherA(stages, GA):
                for cl in ro["callsA"]:
                    if cl.fresh not in stages:  # .fresh holds the stage id
                        continue
                    n = cl.n
                    # call output goes at its slot range [base..base+n)
                    base = 0
                    for c2 in ro["callsA"]:
                        if c2 is cl:
                            break
                        base += c2.n
                    j0 = base // P
                    k = n // P
                    q = qctr[0] % NQ
                    qctr[0] += 1
                    nc.gpsimd.dma_gather(
                        out_ap=GA[:, j0 * 2 * D:(j0 + k) * 2 * D].rearrange(
                            "p (j f) -> p j f", f=2 * D),
                        in_ap=T[cl.base_pair * 2:
                                (cl.base_pair + cl.ext_pair) * 2,
                                :].rearrange("(q x) f -> q (x f)", x=2),
                        idxs_ap=idxAt[:, cl.icol0:cl.icol0 + n // 16],
                        num_idxs=n, num_idxs_reg=n, elem_size=2 * D,
                        single_packet=False, queue_num=q)

            # ---------------- level sweeps ----------------
            GA_ro = None
            if not skip_ro:
                GA_ro = ropool.tile([P, ro["kA"] * 2 * D], f16, tag="GA")
            if NP_ > 0:
                emit_prep(0)
            for i in range(NP_):
                if not skip_ro and i == NP_ - 1 and NP_ >= 2:
                    emit_ro_gatherA({0}, GA_ro)  # oldest: overlap lvl NP_-2
                emit_main(i)
                if i + 1 < NP_:
                    emit_prep(i + 1)
                if not skip_ro and i == NP_ - 1:
                    emit_ro_gatherA({1} if NP_ >= 2 else {0, 1}, GA_ro)

            # ---------------- readout ----------------
            if skip_ro:
                zo = smp.tile([B, 2 * D], f32, tag="outt")
                nc.vector.memset(zo[:, :], 0.0)
                nc.sync.dma_start(out=out_x[:, :], in_=zo[:, :])
                if dump_T:
                    nc.sync.dma_start(out=tdump_x[:, :], in_=T[:, :])
            else:
                emit_ro_gatherA({2}, GA_ro)
                # staging write: stg pair-row p*kA+j <- GA[p, j]
                nc.sync.dma_start(
                    out=stg_t[0:ro["A_tot"], :].rearrange(
                        "(p j) f -> p j f", j=ro["kA"]),
                    in_=GA_ro[:, :].rearrange("p (j f) -> p j f", f=2 * D))
                # phase C gather from staging
                R = ropool.tile([P, NRCH * 2 * D], f16, tag="R")
                q = qctr[0] % NQ
                qctr[0] += 1
                nc.gpsimd.dma_gather(
                    out_ap=R[:, :].rearrange("p (j f) -> p j f", f=2 * D),
                    in_ap=stg_t[:, :],
                    idxs_ap=idxAt[:, ro["icolC"]:
                                  ro["icolC"] + NRCH * P // 16],
                    num_idxs=NRCH * P, num_idxs_reg=NRCH * P,
                    elem_size=2 * D, single_packet=False, queue_num=q)
                # sum pool: psR[g, f] += S16[:, j]ᵀ @ R_par[:, j]
                S16 = sallp.tile([P, NRCH * 16], f16, tag="s16")
                i16t = io16[:, 0:16]
                i16_rep = bass.AP(
                    i16t.tensor, i16t.offset,
                    [i16t.ap[0], [0, NRCH], [1, 16]])
                rgb = rgidt[:, :].to_broadcast([P, NRCH, 16])
                nc.vector.tensor_tensor(
                    out=S16[:, :].rearrange("p (a b) -> p a b", a=NRCH),
                    in0=i16_rep, in1=rgb, op=OP.is_equal)
                ps_sum = psR.tile([B, D], f32, tag="psA")
                for j in range(NRCH):
                    par = ro["chunk_par"][j]
                    nc.tensor.matmul(
                        out=ps_sum[:, :],
                        lhsT=S16[:, j * 16:(j + 1) * 16],
                        rhs=R[:, j * 2 * D + par * D:j * 2 * D + par * D + D],
                        start=(j == 0), stop=(j == NRCH - 1))
                pr = smp.tile([B, 2 * D], f32, tag="pr")
                nc.vector.tensor_copy(out=pr[:, D:2 * D], in_=ps_sum[:, :])
                # max pool per graph: DVE tree over its chunks
                maxT = constp.tile([64, B], f32)
                for g, chs in enumerate(meta["graph_chunks"]):
                    def sl(j):
                        par = ro["chunk_par"][j]
                        return R[:, j * 2 * D + par * D:
                                 j * 2 * D + par * D + D]
                    mxf = smp.tile([P, D], f32, tag="mxf")
                    if len(chs) == 1:
                        nc.vector.tensor_copy(out=mxf[:, :], in_=sl(chs[0]))
                    else:
                        mx = smp.tile([P, D], f16, tag="mx")
                        nc.vector.tensor_tensor(
                            out=mx[:, :], in0=sl(chs[0]), in1=sl(chs[1]),
                            op=OP.max)
                        for j in chs[2:]:
                            nc.vector.tensor_tensor(
                                out=mx[:, :], in0=mx[:, :], in1=sl(j),
                                op=OP.max)
                        nc.vector.tensor_copy(out=mxf[:, :], in_=mx[:, :])
                    pst = psC.tile([64, P], f32, tag="psA")
                    nc.tensor.transpose(
                        out=pst[:, :], in_=mxf[:, :], identity=ident[:, :])
                    nc.vector.reduce_max(
                        out=maxT[:, g:g + 1], in_=pst[:, :], axis=AX.X)
                psmx = psC.tile([B, 64], f32, tag="psA")
                nc.tensor.transpose(
                    out=psmx[:, :], in_=maxT[:, :],
                    identity=ident[0:64, 0:64])
                nc.vector.tensor_copy(out=pr[:, 0:D], in_=psmx[:, :])
                nc.sync.dma_start(
                    out=prin[:, :].rearrange("(h g) f -> g h f", g=B),
                    in_=pr[:, :].rearrange("g (h f) -> g h f", h=2))
                nc.gpsimd.collective_compute(
                    "AllGather", OP.bypass, replica_groups=rg,
                    ins=[prin[:, :]], outs=[prout[:, :]])
                pr3 = prout[:, :].rearrange("(r gg) f -> gg r f", r=C)
                mx8 = smp.tile([B, C * D], f32, tag="mx8")
                sm8 = smp.tile([B, C * D], f32, tag="sm8")
                nc.sync.dma_start(
                    out=mx8[:, :].rearrange("g (r f) -> g r f", f=D),
                    in_=pr3[0:B])
                nc.sync.dma_start(
                    out=sm8[:, :].rearrange("g (r f) -> g r f", f=D),
                    in_=pr3[B:2 * B])
                outt = smp.tile([B, 2 * D], f32, tag="outt")
                for buf, op, o0 in ((mx8, OP.max, 0), (sm8, OP.add, D)):
                    nc.vector.tensor_tensor(
                        out=buf[:, 0:4 * D], in0=buf[:, 0:4 * D],
                        in1=buf[:, 4 * D:8 * D], op=op)
                    nc.vector.tensor_tensor(
                        out=buf[:, 0:2 * D], in0=buf[:, 0:2 * D],
                        in1=buf[:, 2 * D:4 * D], op=op)
                    nc.vector.tensor_tensor(
                        out=outt[:, o0:o0 + D], in0=buf[:, 0:D],
                        in1=buf[:, D:2 * D], op=op)
                nc.sync.dma_start(out=out_x[:, :], in_=outt[:, :])
                if dump_T:
                    nc.sync.dma_start(out=tdump_x[:, :], in_=T[:, :])
    nc.compile()
    return nc


def _in_maps(meta, arrays):
    maps = []
    for c in range(C):
        maps.append(dict(
            tab=arrays["tab"],
            wext=arrays["wext"],
            iota=arrays["iota512"],
            iota16=arrays["iota16"],
            idxA=np.ascontiguousarray(arrays["idxA"][c]),
            cdst=np.ascontiguousarray(arrays["cdst"][c]),
            prt=np.ascontiguousarray(arrays["prt"][c]),
            cnts=np.ascontiguousarray(arrays["cnts"][c]),
            rgid=np.ascontiguousarray(arrays["rgid"][c]),
        ))
    return maps


_LAST_RESULTS = None  # stash for test harness (exec time, trace)


def kernel(**inputs):
    global _LAST_RESULTS
    import os
    meta, arrays = _preprocess(**inputs)
    nc = _build(meta)
    from concourse.bass_utils import run_bass_kernel_spmd
    res = run_bass_kernel_spmd(nc, _in_maps(meta, arrays),
                               core_ids=list(range(C)),
                               trace=bool(os.environ.get("KERNEL_TRACE")))
    _LAST_RESULTS = res
    return np.asarray(res.results[0]["out"])


# revision 59
# speedup vs baseline: 1.8676x; 1.0527x over previous
"""DAG-GNN level-sweep kernel for Trainium2 (8 NeuronCores, Bass/Tile), v2.

Design vs v1:
  - State table T in DRAM is fp16 (tolerance 2e-2 >> fp16's ~3e-4 here);
    gathers fetch PAIRS of consecutive node rows (256B) so the int16
    dma_gather index window covers 65536 node rows, and each level's
    sources fit in a handful of windows.
  - ONE-hop gather per level (no staging round-trip, no permutation
    gather): slots sorted by (window, parity, dst col); chunks are
    window- and parity-pure; the per-chunk selection matmul picks the
    even or odd half of each gathered pair.
  - Old/fresh split: gathers whose sources predate the previous level's
    AllGather are emitted one level early so they overlap the previous
    level's compute + collective; only the small "fresh" gather (sources
    in the immediately preceding level block) sits on the critical path.
  - AllGather moves fp16 (half the bytes of v1).
  - Readout keeps a two-hop (phase A windows -> staging -> graph/parity
    grouped chunks) but phase A overlaps the last levels.
"""

import sys

if "/opt/trn_rl_repo" not in sys.path:
    sys.path.insert(0, "/opt/trn_rl_repo")

import numpy as np

# structural constants of the nn.Module (match reference)
B = 16   # graphs per batch
LF = 12  # forward topological levels
LB = 12  # backward topological levels
D = 64   # hidden dim
C = 8    # NeuronCores
P = 128  # SBUF partitions

# T table header rows (node-row space; pairs are rows (2q, 2q+1))
_H0 = 1        # 9 rows of initial-encoding table, indexed by 3*nt + nip
_MI0 = 10      # fp16-min row, even parity
_MI1 = 11      # fp16-min row, odd parity
_HDR = 12

_WPAIR = 32768   # pair rows addressable by one int16 gather window
_F16MIN = -65504.0


def _ceil(a, b):
    return -(-a // b)


_SPAN = 512   # bulk selection-matrix width per entry (max matmul width)
_EMAX = 2048  # entry col range (fp16-exact integer compares)


class _Lvl:
    __slots__ = (
        "sweep", "l", "Lp", "tb", "nw2", "cnt0",
        "calls", "nch", "ch0", "eh0", "nch_old", "entries", "SP",
    )


class _Call:
    """One dma_gather call: window [base_pair, base_pair+ext_pair) of T."""
    __slots__ = ("base_pair", "ext_pair", "icol0", "n", "fresh")

    def __init__(self, base_pair, ext_pair, icol0, n, fresh):
        self.base_pair = base_pair
        self.ext_pair = ext_pair
        self.icol0 = icol0
        self.n = n
        self.fresh = fresh


def _wrap_idx(a):
    """[C, n] int -> [C, 128, n/16] int16 (16-partition wrap, 8 replicas)."""
    Csz, n = a.shape
    w = a.reshape(Csz, n // 16, 16).transpose(0, 2, 1).astype(np.int16)
    return np.ascontiguousarray(np.tile(w, (1, 8, 1)))


def _preprocess(node_type, num_inverted_predecessors, edge_index,
                forward_level, backward_level, batch,
                W_enc, b_enc, W_f, b_f, W_b, b_b):
    N = int(node_type.shape[0])
    nt = np.asarray(node_type).astype(np.int64)
    nip = np.asarray(num_inverted_predecessors).astype(np.int64)
    fl = np.asarray(forward_level).astype(np.int64)
    bl = np.asarray(backward_level).astype(np.int64)
    bt = np.asarray(batch).astype(np.int64)
    src = np.asarray(edge_index[0]).astype(np.int64)
    dst = np.asarray(edge_index[1]).astype(np.int64)
    code = nt * 3 + nip  # in [0, 9)

    # ---------------- node positions in T ----------------
    posf = np.full(N, -1, np.int64)
    posb = np.full(N, -1, np.int64)
    coref = np.zeros(N, np.int64)
    rankf = np.zeros(N, np.int64)
    coreb = np.zeros(N, np.int64)
    rankb = np.zeros(N, np.int64)

    tbase = _HDR
    lvl_meta = []
    for sweep, lv, pos, core, rank, nl in (
        ("f", fl, posf, coref, rankf, LF),
        ("b", bl, posb, coreb, rankb, LB),
    ):
        for l in range(1, nl):
            idx = np.flatnonzero(lv == l)
            n_l = idx.size
            if n_l == 0:
                lvl_meta.append((sweep, l, 0, tbase))
                continue
            Lp = _ceil(_ceil(n_l, C), P) * P
            c = np.arange(n_l) % C
            r = np.arange(n_l) // C
            pos[idx] = tbase + c * Lp + r
            core[idx] = c
            rank[idx] = r
            lvl_meta.append((sweep, l, Lp, tbase))
            tbase += C * Lp
    T_ROWS = tbase

    # ---------------- per-level plans ----------------
    plans = []
    cdst_blocks = []
    prt_blocks = []
    cnts_blocks = []
    idx_cols = []      # list of [C, 128, k] int16 blocks
    icol0 = 0
    cnt0 = 0
    ch0 = 0
    eh0 = 0
    fl_dst = fl[dst]
    bl_src = bl[src]
    prev_block = None  # (tb, rows) of previous plan's T block
    for (sweep, l, Lp, tb) in lvl_meta:
        if Lp == 0:
            continue
        pl = _Lvl()
        pl.sweep, pl.l, pl.Lp, pl.tb = sweep, l, Lp, tb
        pl.nw2 = Lp // P
        pl.cnt0 = cnt0
        if sweep == "f":
            em = fl_dst == l
            un = dst[em]
            dn = src[em]
            ucore = coref[un]
            urank = rankf[un]
            gat = (fl[dn] >= 1) & (fl[dn] < l)
            gpos_all = posf[dn]
        else:
            em = bl_src == l
            un = src[em]
            dn = dst[em]
            ucore = coreb[un]
            urank = rankb[un]
            upd_b = (bl[dn] >= 1) & (bl[dn] < l)
            upd_f = (~upd_b) & (fl[dn] >= 1)
            gat = upd_b | upd_f
            gpos_all = np.where(upd_b, posb[dn], posf[dn])

        # counts: initial-valued sources by code, plus total indegree (bias)
        cnt = np.zeros((C, Lp, 10), np.float32)
        i0 = ~gat
        np.add.at(cnt, (ucore[i0], urank[i0], code[dn[i0]]), 1.0)
        np.add.at(cnt, (ucore, urank, 9), 1.0)
        cnts_blocks.append(np.ascontiguousarray(
            cnt.transpose(0, 2, 1)).astype(np.float16))
        cnt0 += Lp

        gc = ucore[gat]
        gr = urank[gat]        # dst col (rank == col)
        gp = gpos_all[gat]     # source node row in T
        if gp.size == 0:
            pl.calls, pl.nch, pl.ch0, pl.eh0 = [], 0, ch0, eh0
            pl.nch_old, pl.entries, pl.SP = 0, [], _SPAN
            plans.append(pl)
            prev_block = (tb, C * Lp)
            continue

        # window id: old region [0, fs) in _WPAIR-pair windows; fresh = prev
        fs = prev_block[0]
        fresh_rows = prev_block[1]
        n_oldwin = _ceil(fs // 2, _WPAIR) if fs > 0 else 0
        wid = np.where(gp < fs, (gp // 2) // _WPAIR, n_oldwin)
        par = gp % 2
        poff = np.where(gp < fs, (gp // 2) % _WPAIR, gp // 2 - fs // 2)

        # group sizes per window (parity folded into the mask): uniform
        gsz = np.zeros(n_oldwin + 1, np.int64)
        for w in range(n_oldwin + 1):
            mx = 0
            for c in range(C):
                mx = max(mx, int(((gc == c) & (wid == w)).sum()))
            gsz[w] = _ceil(mx, P) * P if mx else 0
        S_tot = int(gsz.sum())
        nch = S_tot // P

        # slot arrays [C, S_tot]
        off_sl = np.zeros((C, S_tot), np.int64)   # pair offset in window
        col_sl = np.full((C, S_tot), -1, np.int64)  # dst col (-1 padding)
        par_sl = np.zeros((C, S_tot), np.int64)
        gstart = np.concatenate([[0], np.cumsum(gsz)])[:-1]
        for c in range(C):
            m_c = gc == c
            order = np.lexsort((gr[m_c], wid[m_c]))
            wo = wid[m_c][order]
            oo = poff[m_c][order]
            ro = gr[m_c][order]
            qo = par[m_c][order]
            cnts_g = np.bincount(wo, minlength=n_oldwin + 1)
            st = np.concatenate([[0], np.cumsum(cnts_g)])[:-1]
            seq = np.arange(wo.size) - st[wo]
            slot = gstart[wo] + seq
            off_sl[c, slot] = oo
            col_sl[c, slot] = ro
            par_sl[c, slot] = qo

        # gather calls: one per window with slots
        calls = []
        nch_old = 0
        for w in range(n_oldwin + 1):
            n_g = int(gsz[w])
            if n_g == 0:
                continue
            if w < n_oldwin:
                base = w * _WPAIR
                ext = min(_WPAIR, fs // 2 - base)
                fresh = False
                nch_old += n_g // P
            else:
                base = fs // 2
                ext = fresh_rows // 2
                fresh = True
            a = int(gstart[w])
            blk = off_sl[:, a:a + n_g]
            idx_cols.append(_wrap_idx(blk))
            calls.append(_Call(base, ext, icol0, n_g, fresh))
            icol0 += n_g // 16
        pl.calls = calls
        pl.nch_old = nch_old

        # per-chunk entries (col range <= _EMAX each) with exact-span pieces
        raw = []       # (j, b, emax, cdv)
        for j in range(nch):
            v = col_sl[:, j * P:(j + 1) * P]   # [C, P]
            vv = v[v >= 0]
            if vv.size == 0:
                continue
            lo, hi = int(vv.min()), int(vv.max())
            for b in range(lo, hi + 1, _EMAX):
                sel = (v >= b) & (v < b + _EMAX)
                if not sel.any():
                    continue
                emax = int(v[sel].max()) - b
                cdv = np.where(sel, v - b, -1).astype(np.float16)
                raw.append((j, b, emax, cdv))
        SP = min(_SPAN, _ceil(max(e for (_, _, e, _) in raw) + 1, P) * P)
        pl.SP = SP
        entries = []   # (j, c0_abs, pieces, ecol)
        ecd_cols = []  # per-entry cdst column [C, P]
        for (j, b, emax, cdv) in raw:
            pc = []
            k = 0
            span = emax + 1
            while span > 0:
                w_k = min(SP, span)
                pc.append((k, w_k))
                span -= w_k
                k += 1
            entries.append((j, b, pc, len(ecd_cols)))
            ecd_cols.append(cdv)
        nE = len(ecd_cols)
        cdst_blocks.append(np.ascontiguousarray(
            np.stack(ecd_cols, axis=2)) if nE
            else np.full((C, P, 1), -1.0, np.float16))
        prt_blocks.append(np.ascontiguousarray(
            par_sl.reshape(C, nch, P).transpose(0, 2, 1)).astype(np.float16))
        pl.nch, pl.ch0, pl.eh0 = nch, ch0, eh0
        pl.entries = entries
        ch0 += nch
        eh0 += max(nE, 1)
        plans.append(pl)
        prev_block = (tb, C * Lp)

    TOTCH = max(1, ch0)
    TOTE = max(1, eh0)
    CNT_TOT = cnt0
    cdst_all = (np.concatenate(cdst_blocks, axis=2) if cdst_blocks
                else np.full((C, P, 1), -1.0, np.float16))
    prt_all = (np.concatenate(prt_blocks, axis=2) if prt_blocks
               else np.zeros((C, P, 1), np.float16))
    if cdst_all.shape[2] < TOTE:
        pad = TOTE - cdst_all.shape[2]
        cdst_all = np.concatenate(
            [cdst_all, np.full((C, P, pad), -1.0, np.float16)], axis=2)
    if prt_all.shape[2] < TOTCH:
        pad = TOTCH - prt_all.shape[2]
        prt_all = np.concatenate(
            [prt_all, np.zeros((C, P, pad), np.float16)], axis=2)
    cnts_all = np.concatenate(cnts_blocks, axis=2)

    # ---------------- readout layout ----------------
    last_tb = plans[-1].tb
    last_rows = C * plans[-1].Lp
    mid_tb = plans[-2].tb
    onodes = np.flatnonzero(nt == 1)
    og = bt[onodes]
    fpos = np.where(bl[onodes] >= 1, posb[onodes],
                    np.where(fl[onodes] >= 1, posf[onodes],
                             _H0 + code[onodes]))
    oc = np.arange(onodes.size) % C  # core assignment

    # --- phase A: slots grouped per window; stage order = arrival order
    # stage 0: rows < mid_tb (oldest); 1: [mid_tb, last_tb); 2: last block
    n_oldwinA = _ceil(mid_tb // 2, _WPAIR)
    widA = np.where(fpos < mid_tb, (fpos // 2) // _WPAIR,
                    np.where(fpos < last_tb, n_oldwinA, n_oldwinA + 1))
    basesA = [w * _WPAIR for w in range(n_oldwinA)] + \
        [mid_tb // 2, last_tb // 2]
    extsA = [min(_WPAIR, mid_tb // 2 - w * _WPAIR)
             for w in range(n_oldwinA)] + \
        [(last_tb - mid_tb) // 2, last_rows // 2]
    stageA = [0] * n_oldwinA + [1, 2]
    nwinA = n_oldwinA + 2
    # per (window) sizes uniform across cores; slot 0 of window 0 reserved
    # for the fp16-min pair (pair row 5) used as padding target.
    poffA = fpos // 2 - np.asarray(basesA, np.int64)[widA]
    gszA = np.zeros(nwinA, np.int64)
    for w in range(nwinA):
        mx = 0
        for c in range(C):
            mx = max(mx, int(((oc == c) & (widA == w)).sum()))
        mx += (1 if w == 0 else 0)  # reserved pad slot
        gszA[w] = _ceil(mx, P) * P if mx else 0
    A_tot = int(gszA.sum())
    kA = A_tot // P
    gstartA = np.concatenate([[0], np.cumsum(gszA)])[:-1]
    offA = np.zeros((C, A_tot), np.int64)
    offA[:, 0] = _MI0 // 2  # pad pair (both halves fp16-min)
    # staged row of phase-A slot q (GA[p, j] -> stg row p*kA+j)
    stg_row = (np.arange(A_tot) % P) * kA + np.arange(A_tot) // P
    posA = np.zeros((C, onodes.size), np.int64)  # node -> phase-A slot
    for c in range(C):
        m_c = oc == c
        order = np.argsort(widA[m_c], kind="stable")
        wo = widA[m_c][order]
        oo = poffA[m_c][order]
        cnts_g = np.bincount(wo, minlength=nwinA)
        st = np.concatenate([[0], np.cumsum(cnts_g)])[:-1]
        seq = np.arange(wo.size) - st[wo]
        slot = gstartA[wo] + seq + (wo == 0)  # skip reserved slot 0
        offA[c, slot] = oo
        posA[c, np.flatnonzero(m_c)[order]] = slot
    ro_callsA = []
    icolA_blocks = []
    for w in range(nwinA):
        n_g = int(gszA[w])
        if n_g == 0:
            continue
        a = int(gstartA[w])
        icolA_blocks.append(offA[:, a:a + n_g])
        ro_callsA.append(_Call(basesA[w], extsA[w], icol0, n_g, stageA[w]))
        icol0 += n_g // 16
    for blk in icolA_blocks:
        idx_cols.append(_wrap_idx(blk))

    # --- phase C: final chunks grouped by (graph, parity)
    parR = fpos % 2
    kg2 = np.zeros((B, 2), np.int64)
    for g in range(B):
        for q in range(2):
            mx = 0
            for c in range(C):
                mx = max(mx, int(((oc == c) & (og == g) &
                                  (parR == q)).sum()))
            kg2[g, q] = _ceil(max(mx, 1), P) * P
    R_tot = int(kg2.sum())
    NRCH = R_tot // P
    gstartR = np.concatenate([[0], np.cumsum(kg2.reshape(-1))])[:-1]
    gstartR = gstartR.reshape(B, 2)
    idxC = np.zeros((C, R_tot), np.int64)
    idxC[:, :] = int(stg_row[0])  # padding -> staged fp16-min pair
    rgid = np.full((C, R_tot), -1.0, np.float32)
    chunk_parR = np.zeros(NRCH, np.int64)
    graph_chunks = []  # per graph: list of chunk ids
    for g in range(B):
        chs = []
        for q in range(2):
            a, n_g = int(gstartR[g, q]), int(kg2[g, q])
            chunk_parR[a // P:(a + n_g) // P] = q
            chs.extend(range(a // P, (a + n_g) // P))
        graph_chunks.append(chs)
    for c in range(C):
        m_c = oc == c
        order = np.lexsort((parR[m_c], og[m_c]))
        go = og[m_c][order]
        qo = parR[m_c][order]
        so = posA[c, m_c][order]
        grp = go * 2 + qo
        cnts_g = np.bincount(grp, minlength=B * 2)
        st = np.concatenate([[0], np.cumsum(cnts_g)])[:-1]
        seq = np.arange(go.size) - st[grp]
        slot = gstartR[go, qo] + seq
        idxC[c, slot] = stg_row[so]
        rgid[c, slot] = go
    icolC = icol0
    idx_cols.append(_wrap_idx(idxC))
    icol0 += R_tot // 16
    rgid_arr = np.ascontiguousarray(
        rgid.reshape(C, NRCH, P).transpose(0, 2, 1)).astype(np.float16)

    # ---------------- weight-derived constants ----------------
    W_enc = np.asarray(W_enc, np.float32)
    b_enc = np.asarray(b_enc, np.float32)
    W_f = np.asarray(W_f, np.float32)
    b_f = np.asarray(b_f, np.float32)
    W_b = np.asarray(W_b, np.float32)
    b_b = np.asarray(b_b, np.float32)
    h0_tab = np.zeros((9, D), np.float32)
    for cc in range(9):
        h0_tab[cc] = (cc // 3) * W_enc[0] + (cc % 3) * W_enc[1] + b_enc
    tab = np.zeros((_HDR, D), np.float32)
    tab[_H0:_H0 + 9] = h0_tab
    tab[_MI0] = _F16MIN
    tab[_MI1] = _F16MIN
    wf_ext = np.concatenate([W_f, h0_tab @ W_f, b_f[None, :]], axis=0)
    wb_ext = np.concatenate([W_b, h0_tab @ W_b, b_b[None, :]], axis=0)
    wext = np.ascontiguousarray(np.concatenate([wf_ext, wb_ext], axis=1))

    iota512 = np.ascontiguousarray(
        np.tile(np.arange(512, dtype=np.float16), (P, 1)))
    iota16 = np.ascontiguousarray(
        np.tile(np.arange(16, dtype=np.float16), (P, 1)))

    idxA_all = np.concatenate(idx_cols, axis=2)
    meta = dict(
        plans=plans, graph_chunks=graph_chunks,
        T_ROWS=T_ROWS, AG_ROWS=max(1, CNT_TOT), TOTCH=TOTCH, TOTE=TOTE,
        CNT_TOT=max(1, CNT_TOT), NRCH=NRCH, AW=idxA_all.shape[2],
        ro=dict(callsA=ro_callsA, kA=kA, A_tot=A_tot, icolC=icolC,
                chunk_par=chunk_parR.tolist()),
    )
    arrays = dict(
        tab=tab.astype(np.float16), wext=wext.astype(np.float16),
        iota512=iota512, iota16=iota16, idxA=idxA_all,
        cdst=cdst_all, prt=prt_all, cnts=cnts_all, rgid=rgid_arr,
    )
    return meta, arrays


# ---------------------------------------------------------------------------
# pure-numpy execution of the plan (host self-check / debugging)
# ---------------------------------------------------------------------------

def _sim_gather(T16, arrays, c, calls, n_slots):
    """numpy mirror of the device one-hop pair gather -> [n_slots, 2, D]."""
    idxA = arrays["idxA"][c][0:16, :]
    out = np.zeros((n_slots, 2, D), np.float32)
    Tp = T16.reshape(-1, 2, D)
    base = 0
    for cl in calls:
        cols = slice(cl.icol0, cl.icol0 + cl.n // 16)
        off = idxA[:, cols].T.reshape(-1)[:cl.n].astype(np.int64)
        assert off.max(initial=0) < cl.ext_pair
        out[base:base + cl.n] = Tp[cl.base_pair + off]
        base += cl.n
    assert base == n_slots
    return out


def _simulate_plan(meta, arrays, fp16=True, return_T=False):
    def rnd(x):
        return (np.clip(x, _F16MIN, -_F16MIN).astype(np.float16)
                .astype(np.float32) if fp16 else x)

    T = np.zeros((meta["T_ROWS"], D), np.float32)
    T[0:_HDR] = arrays["tab"].astype(np.float32)
    wext = arrays["wext"].astype(np.float32)
    for pl in meta["plans"]:
        wmat = wext[:, 0:D] if pl.sweep == "f" else wext[:, D:2 * D]
        blocks = []
        for c in range(C):
            lhs = np.zeros((74, pl.Lp), np.float32)
            lhs[64:74] = arrays["cnts"][c, :, pl.cnt0:pl.cnt0 + pl.Lp]
            if pl.nch > 0:
                prv = arrays["prt"][c][:, pl.ch0:pl.ch0 + pl.nch].astype(
                    np.float32)
                Gp = _sim_gather(T, arrays, c, pl.calls, pl.nch * P)
                # G[p, j, 2*D] = slot j*P+p (full pair), parity-masked
                G = Gp.reshape(pl.nch, P, 2 * D).transpose(1, 0, 2).copy()
                half = (np.arange(2 * D) // D)[None, None, :]
                G *= (prv[:, :, None] == half)
                ps = np.zeros((2 * D, pl.Lp), np.float32)
                for (j, b, pieces, ecol) in pl.entries:
                    cdv = arrays["cdst"][c][:, pl.eh0 + ecol].astype(
                        np.float32)
                    for (k, w_k) in pieces:
                        a = b + k * pl.SP
                        S = (cdv[:, None] ==
                             (np.arange(w_k) + k * pl.SP)[None, :])
                        ps[:, a:a + w_k] += G[:, j, :].T @ S.astype(
                            np.float32)
                lhs[0:64] = rnd(ps[0:D] + ps[D:2 * D])
            blocks.append(rnd(lhs.T @ wmat))  # [Lp, D]
        T[pl.tb:pl.tb + C * pl.Lp] = np.concatenate(blocks, axis=0)
    # readout
    ro = meta["ro"]
    NRCH = meta["NRCH"]
    maxp = np.full((B, D), _F16MIN, np.float32)
    sump = np.zeros((B, D), np.float32)
    for c in range(C):
        GA = _sim_gather(T, arrays, c, ro["callsA"], ro["A_tot"])
        # staged row of slot q = (q % P) * kA + q // P
        stg = np.zeros((ro["A_tot"], 2, D), np.float32)
        q = np.arange(ro["A_tot"])
        stg[(q % P) * ro["kA"] + q // P] = GA
        idxC = arrays["idxA"][c][0:16,
                                 ro["icolC"]:ro["icolC"] + NRCH * P // 16]
        off = idxC.T.reshape(-1).astype(np.int64)
        R_lin = stg[off]  # [NRCH*P, 2, D]
        R = R_lin.reshape(NRCH, P, 2, D)
        gid = arrays["rgid"][c]  # [P, NRCH]
        for g, chs in enumerate(meta["graph_chunks"]):
            for j in chs:
                sl = R[j, :, ro["chunk_par"][j], :]  # [P, D]
                maxp[g] = np.maximum(maxp[g], sl.max(axis=0))
                msk = (gid[:, j] == g).astype(np.float32)
                sump[g] += msk @ sl
    out = np.concatenate([maxp, sump], axis=1)
    return (out, T) if return_T else out


# ---------------------------------------------------------------------------
# Bass program
# ---------------------------------------------------------------------------

def _build(meta):
    import os

    import concourse.bass as bass
    import concourse.mybir as mybir
    from concourse import bacc, tile
    from concourse.masks import make_identity

    f32 = mybir.dt.float32
    f16 = mybir.dt.float16
    i16 = mybir.dt.int16
    AX = mybir.AxisListType
    OP = mybir.AluOpType

    TOTCH, CNT_TOT, NRCH = meta["TOTCH"], meta["CNT_TOT"], meta["NRCH"]
    TOTE = meta["TOTE"]
    AW = meta["AW"]
    plans = meta["plans"]
    NQ = 4  # SWDGE queues, round-robin over gather calls

    nc = bacc.Bacc(None, num_devices=C, num_swdge_queues=NQ)
    tab_x = nc.dram_tensor("tab", [_HDR, D], f16, kind="ExternalInput")
    wext_x = nc.dram_tensor("wext", [74, 2 * D], f16, kind="ExternalInput")
    iota_x = nc.dram_tensor("iota", [P, 512], f16, kind="ExternalInput")
    io16_x = nc.dram_tensor("iota16", [P, 16], f16, kind="ExternalInput")
    idxA_x = nc.dram_tensor("idxA", [P, AW], i16, kind="ExternalInput")
    cdst_x = nc.dram_tensor("cdst", [P, TOTE], f16, kind="ExternalInput")
    prt_x = nc.dram_tensor("prt", [P, TOTCH], f16, kind="ExternalInput")
    cnts_x = nc.dram_tensor("cnts", [10, CNT_TOT], f16, kind="ExternalInput")
    rgid_x = nc.dram_tensor("rgid", [P, NRCH], f16, kind="ExternalInput")
    out_x = nc.dram_tensor("out", [B, 2 * D], f32, kind="ExternalOutput")

    T = nc.dram_tensor("T", [meta["T_ROWS"], D], f16, addr_space="Shared")
    dump_T = bool(os.environ.get("KERNEL_DUMP_T"))
    tdump_x = (nc.dram_tensor("tdump", [meta["T_ROWS"], D], f16,
                              kind="ExternalOutput") if dump_T else None)
    agin = nc.dram_tensor("agin", [meta["AG_ROWS"], D], f16)
    ro = meta["ro"]
    stg_t = nc.dram_tensor("stg", [max(ro["A_tot"], P), 2 * D], f16)
    prin = nc.dram_tensor("prin", [2 * B, D], f32)
    prout = nc.dram_tensor("prout", [C * 2 * B, D], f32, addr_space="Shared")

    rg = [list(range(C))]
    qctr = [0]

    max_lv = int(os.environ.get("KERNEL_MAX_LEVELS", "99"))
    skip_ro = bool(os.environ.get("KERNEL_SKIP_READOUT"))
    plans = plans[:max_lv]
    NP_ = len(plans)

    with tile.TileContext(nc) as tc:
        with (
            tc.tile_pool(name="const", bufs=1) as constp,
            tc.tile_pool(name="g", bufs=2) as gpool,
            tc.tile_pool(name="ro", bufs=1) as ropool,
            tc.tile_pool(name="sall", bufs=2) as sallp,
            tc.tile_pool(name="sx", bufs=4) as sxp,
            tc.tile_pool(name="lhs", bufs=2) as lhsp,
            tc.tile_pool(name="msg", bufs=2) as msgp,
            tc.tile_pool(name="small", bufs=1) as smp,
            tc.tile_pool(name="psA", bufs=1, space="PSUM") as psA,
            tc.tile_pool(name="psB", bufs=1, space="PSUM") as psB,
        ):
            psC = psA  # readout PSUM tiles reuse the (dead) psA slot
            psR = psA
            # ---------------- setup ----------------
            stg0t = constp.tile([_HDR, D], f16)
            nc.sync.dma_start(out=stg0t[:, :], in_=tab_x[:, :])
            nc.sync.dma_start(out=T[0:_HDR, :], in_=stg0t[:, :])
            wt = constp.tile([74, 2 * D], f16)
            nc.sync.dma_start(out=wt[:, :], in_=wext_x[:, :])
            iot = constp.tile([P, 512], f16)
            nc.sync.dma_start(out=iot[:, :], in_=iota_x[:, :])
            io16 = constp.tile([P, 16], f16)
            nc.sync.dma_start(out=io16[:, :], in_=io16_x[:, :])
            cdstt = constp.tile([P, TOTE], f16)
            nc.sync.dma_start(out=cdstt[:, :], in_=cdst_x[:, :])
            cdst32 = constp.tile([P, TOTE], f32)
            nc.scalar.activation(
                cdst32[:, :], cdstt[:, :],
                mybir.ActivationFunctionType.Copy)
            prtt = constp.tile([P, TOTCH], f16)
            nc.sync.dma_start(out=prtt[:, :], in_=prt_x[:, :])
            hiota = constp.tile([P, 2 * D], f16)
            nc.vector.tensor_scalar(
                hiota[:, :], iot[:, 0:2 * D], float(D), None, OP.is_ge)
            idxAt = constp.tile([P, AW], i16)
            nc.sync.dma_start(out=idxAt[:, :], in_=idxA_x[:, :])
            rgidt = constp.tile([P, NRCH], f16)
            nc.sync.dma_start(out=rgidt[:, :], in_=rgid_x[:, :])
            ident = constp.tile([P, P], f32)
            make_identity(nc, ident[:, :])

            def emit_gather(calls, which, G, frontier):
                """Emit dma_gather calls; which selects fresh/old/all."""
                base = 0
                for cl in calls:
                    n = cl.n
                    j0 = base // P
                    k = n // P
                    base += n
                    if which == "fresh" and not cl.fresh:
                        continue
                    if which == "old" and cl.fresh:
                        continue
                    wrows = min(cl.ext_pair, frontier // 2 - cl.base_pair)
                    q = qctr[0] % NQ
                    qctr[0] += 1
                    nc.gpsimd.dma_gather(
                        out_ap=G[:, j0 * 2 * D:(j0 + k) * 2 * D].rearrange(
                            "p (j f) -> p j f", f=2 * D),
                        in_ap=T[cl.base_pair * 2:
                                (cl.base_pair + wrows) * 2, :].rearrange(
                            "(q x) f -> q (x f)", x=2),
                        idxs_ap=idxAt[:, cl.icol0:cl.icol0 + n // 16],
                        num_idxs=n, num_idxs_reg=n, elem_size=2 * D,
                        single_packet=False, queue_num=q)

            state = {}

            def emit_aggs(pl, st, old):
                G, ps, Sall = st["G"], st["ps"], st["Sall"]
                SP = pl.SP
                for (j, b, pieces, ecol) in pl.entries:
                    if (j < pl.nch_old) != old:
                        continue
                    for (k, w_k) in pieces:
                        a = b + k * SP
                        if k == 0:
                            S_ap = Sall[:, ecol * SP:ecol * SP + w_k]
                        else:
                            S2 = sxp.tile([P, _SPAN], f16, tag="S2")
                            nc.vector.tensor_scalar(
                                S2[:, 0:w_k], iot[:, 0:w_k],
                                cdst32[:, pl.eh0 + ecol:pl.eh0 + ecol + 1],
                                float(-k * SP), OP.subtract, OP.is_equal)
                            S_ap = S2[:, 0:w_k]
                        nc.tensor.matmul(
                            out=ps[:, a:a + w_k],
                            lhsT=G[:, j * 2 * D:(j + 1) * 2 * D],
                            rhs=S_ap,
                            start=False, stop=False, skip_group_check=True)

            def emit_prep(i):
                """S build (DVE), counts DMA, old gathers, old aggs."""
                pl = plans[i]
                Lp, nch = pl.Lp, pl.nch
                lhs = lhsp.tile([74, Lp], f16, tag="lhs")
                lhsO = lhsp.tile([64, Lp], f16, tag="lhsO")
                nc.sync.dma_start(
                    out=lhs[64:74, :],
                    in_=cnts_x[:, pl.cnt0:pl.cnt0 + Lp])
                st = dict(lhs=lhs, lhsO=lhsO)
                if nch > 0:
                    G = gpool.tile([P, nch * 2 * D], f16, tag="G")
                    st["G"] = G
                    emit_gather(pl.calls, "old", G, meta["T_ROWS"] * 2)
                    # parity masks for all chunks + selection matrices
                    M = sallp.tile([P, nch * 2 * D], f16, tag="M")
                    ih = hiota[:, 0:2 * D]
                    ih_rep = bass.AP(
                        ih.tensor, ih.offset,
                        [ih.ap[0], [0, nch], [1, 2 * D]])
                    prb = prtt[:, pl.ch0:pl.ch0 + nch].to_broadcast(
                        [P, nch, 2 * D])
                    nc.vector.tensor_tensor(
                        out=M[:, :].rearrange("p (a b) -> p a b", a=nch),
                        in0=ih_rep, in1=prb, op=OP.is_equal)
                    st["M"] = M
                    nE = max(len(pl.entries), 1)
                    SP = pl.SP
                    Sall = sallp.tile([P, nE * SP], f16, tag="S")
                    st["Sall"] = Sall
                    ia = iot[:, 0:SP]
                    iota_rep = bass.AP(
                        ia.tensor, ia.offset,
                        [ia.ap[0], [0, nE], [1, SP]])
                    cdb = cdstt[:, pl.eh0:pl.eh0 + nE].to_broadcast(
                        [P, nE, SP])
                    nc.vector.tensor_tensor(
                        out=Sall[:, :].rearrange("p (a b) -> p a b", a=nE),
                        in0=iota_rep, in1=cdb, op=OP.is_equal)
                    ps = psA.tile([2 * D, Lp], f32, tag="psA")
                    st["ps"] = ps
                    if i == 0:
                        nc.vector.memset(ps[:, :], 0.0)
                    else:
                        nc.scalar.activation(
                            ps[:, :], ps[:, :],
                            mybir.ActivationFunctionType.Copy, 0.0, 0.0)
                    if pl.nch_old > 0:
                        nc.vector.tensor_tensor(
                            out=G[:, 0:pl.nch_old * 2 * D],
                            in0=G[:, 0:pl.nch_old * 2 * D],
                            in1=M[:, 0:pl.nch_old * 2 * D], op=OP.mult)
                        emit_aggs(pl, st, True)
                state[i] = st

            def emit_main(i):
                pl = plans[i]
                Lp, nch, nw2 = pl.Lp, pl.nch, pl.nw2
                st = state.pop(i)
                lhs = st["lhs"]
                lhsO = st["lhsO"]
                if nch > 0:
                    G, M, ps = st["G"], st["M"], st["ps"]
                    if nch > pl.nch_old:
                        emit_gather(pl.calls, "fresh", G, pl.tb)
                        nc.vector.tensor_tensor(
                            out=G[:, pl.nch_old * 2 * D:nch * 2 * D],
                            in0=G[:, pl.nch_old * 2 * D:nch * 2 * D],
                            in1=M[:, pl.nch_old * 2 * D:nch * 2 * D],
                            op=OP.mult)
                        emit_aggs(pl, st, False)
                    nc.scalar.activation(
                        lhs[0:64, :], ps[0:D, :],
                        mybir.ActivationFunctionType.Copy)
                    nc.scalar.activation(
                        lhsO[:, :], ps[D:2 * D, :],
                        mybir.ActivationFunctionType.Copy)
                else:
                    nc.vector.memset(lhs[0:64, :], 0.0)

                msg = msgp.tile([P, nw2 * D], f16, tag="msg")
                wcol = 0 if pl.sweep == "f" else D
                psW = psB.tile([P, nw2 * D], f32, tag="psW")
                for w2 in range(nw2):
                    nc.tensor.matmul(
                        out=psW[:, w2 * D:(w2 + 1) * D],
                        lhsT=lhs[:, w2 * P:(w2 + 1) * P],
                        rhs=wt[:, wcol:wcol + D],
                        start=True, stop=(nch == 0))
                    if nch > 0:
                        nc.tensor.matmul(
                            out=psW[:, w2 * D:(w2 + 1) * D],
                            lhsT=lhsO[:, w2 * P:(w2 + 1) * P],
                            rhs=wt[0:64, wcol:wcol + D],
                            start=False, stop=True)
                nc.scalar.activation(
                    msg[:, :], psW[:, :], mybir.ActivationFunctionType.Copy)
                nc.sync.dma_start(
                    out=agin[pl.cnt0:pl.cnt0 + Lp, :].rearrange(
                        "(j p) f -> p j f", p=P),
                    in_=msg[:, :].rearrange("p (j f) -> p j f", f=D))
                nc.gpsimd.collective_compute(
                    "AllGather", OP.bypass, replica_groups=rg,
                    ins=[agin[pl.cnt0:pl.cnt0 + Lp, :]],
                    outs=[T[pl.tb:pl.tb + C * pl.Lp, :]])

            def emit_ro_gatherA(stages, GA):
                for cl in ro["callsA"]:
                    if cl.fresh not in stages:  # .fresh holds the stage id
                        continue
                    n = cl.n
                    # call output goes at its slot range [base..base+n)
                    base = 0
                    for c2 in ro["callsA"]:
                        if c2 is cl:
                            break
                        base += c2.n
                    j0 = base // P
                    k = n // P
                    q = qctr[0] % NQ
                    qctr[0] += 1
                    nc.gpsimd.dma_gather(
                        out_ap=GA[:, j0 * 2 * D:(j0 + k) * 2 * D].rearrange(
                            "p (j f) -> p j f", f=2 * D),
                        in_ap=T[cl.base_pair * 2:
                                (cl.base_pair + cl.ext_pair) * 2,
                                :].rearrange("(q x) f -> q (x f)", x=2),
                        idxs_ap=idxAt[:, cl.icol0:cl.icol0 + n // 16],
                        num_idxs=n, num_idxs_reg=n, elem_size=2 * D,
                        single_packet=False, queue_num=q)

            # ---------------- level sweeps ----------------
            GA_ro = None
            if not skip_ro:
                GA_ro = ropool.tile([P, ro["kA"] * 2 * D], f16, tag="GA")
            if NP_ > 0:
                emit_prep(0)
            for i in range(NP_):
                if not skip_ro and i == NP_ - 1 and NP_ >= 2:
                    emit_ro_gatherA({0}, GA_ro)  # oldest: overlap lvl NP_-2
                emit_main(i)
                if i + 1 < NP_:
                    emit_prep(i + 1)
                if not skip_ro and i == NP_ - 1:
                    emit_ro_gatherA({1} if NP_ >= 2 else {0, 1}, GA_ro)

            # ---------------- readout ----------------
            if skip_ro:
                zo = smp.tile([B, 2 * D], f32, tag="outt")
                nc.vector.memset(zo[:, :], 0.0)
                nc.sync.dma_start(out=out_x[:, :], in_=zo[:, :])
                if dump_T:
                    nc.sync.dma_start(out=tdump_x[:, :], in_=T[:, :])
            else:
                emit_ro_gatherA({2}, GA_ro)
                # staging write: stg pair-row p*kA+j <- GA[p, j]
                nc.sync.dma_start(
                    out=stg_t[0:ro["A_tot"], :].rearrange(
                        "(p j) f -> p j f", j=ro["kA"]),
                    in_=GA_ro[:, :].rearrange("p (j f) -> p j f", f=2 * D))
                # phase C gather from staging
                R = ropool.tile([P, NRCH * 2 * D], f16, tag="R")
                q = qctr[0] % NQ
                qctr[0] += 1
                nc.gpsimd.dma_gather(
                    out_ap=R[:, :].rearrange("p (j f) -> p j f", f=2 * D),
                    in_ap=stg_t[:, :],
                    idxs_ap=idxAt[:, ro["icolC"]:
                                  ro["icolC"] + NRCH * P // 16],
                    num_idxs=NRCH * P, num_idxs_reg=NRCH * P,
                    elem_size=2 * D, single_packet=False, queue_num=q)
                # sum pool: psR[g, f] += S16[:, j]ᵀ @ R_par[:, j]
                S16 = sallp.tile([P, NRCH * 16], f16, tag="s16")
                i16t = io16[:, 0:16]
                i16_rep = bass.AP(
                    i16t.tensor, i16t.offset,
                    [i16t.ap[0], [0, NRCH], [1, 16]])
                rgb = rgidt[:, :].to_broadcast([P, NRCH, 16])
                nc.vector.tensor_tensor(
                    out=S16[:, :].rearrange("p (a b) -> p a b", a=NRCH),
                    in0=i16_rep, in1=rgb, op=OP.is_equal)
                ps_sum = psR.tile([B, D], f32, tag="psA")
                for j in range(NRCH):
                    par = ro["chunk_par"][j]
                    nc.tensor.matmul(
                        out=ps_sum[:, :],
                        lhsT=S16[:, j * 16:(j + 1) * 16],
                        rhs=R[:, j * 2 * D + par * D:j * 2 * D + par * D + D],
                        start=(j == 0), stop=(j == NRCH - 1))
                pr = smp.tile([B, 2 * D], f32, tag="pr")
                nc.vector.tensor_copy(out=pr[:, D:2 * D], in_=ps_sum[:, :])
                # max pool per graph: DVE tree over its chunks
                maxT = constp.tile([64, B], f32)
                for g, chs in enumerate(meta["graph_chunks"]):
                    def sl(j):
                        par = ro["chunk_par"][j]
                        return R[:, j * 2 * D + par * D:
                                 j * 2 * D + par * D + D]
                    mxf = smp.tile([P, D], f32, tag="mxf")
                    if len(chs) == 1:
                        nc.vector.tensor_copy(out=mxf[:, :], in_=sl(chs[0]))
                    else:
                        mx = smp.tile([P, D], f16, tag="mx")
                        nc.vector.tensor_tensor(
                            out=mx[:, :], in0=sl(chs[0]), in1=sl(chs[1]),
                            op=OP.max)
                        for j in chs[2:]:
                            nc.vector.tensor_tensor(
                                out=mx[:, :], in0=mx[:, :], in1=sl(j),
                                op=OP.max)
                        nc.vector.tensor_copy(out=mxf[:, :], in_=mx[:, :])
                    pst = psC.tile([64, P], f32, tag="psA")
                    nc.tensor.transpose(
                        out=pst[:, :], in_=mxf[:, :], identity=ident[:, :])
                    nc.vector.reduce_max(
                        out=maxT[:, g:g + 1], in_=pst[:, :], axis=AX.X)
                psmx = psC.tile([B, 64], f32, tag="psA")
                nc.tensor.transpose(
                    out=psmx[:, :], in_=maxT[:, :],
                    identity=ident[0:64, 0:64])
                nc.vector.tensor_copy(out=pr[:, 0:D], in_=psmx[:, :])
                nc.sync.dma_start(
                    out=prin[:, :].rearrange("(h g) f -> g h f", g=B),
                    in_=pr[:, :].rearrange("g (h f) -> g h f", h=2))
                nc.gpsimd.collective_compute(
                    "AllGather", OP.bypass, replica_groups=rg,
                    ins=[prin[:, :]], outs=[prout[:, :]])
                pr3 = prout[:, :].rearrange("(r gg) f -> gg r f", r=C)
                mx8 = smp.tile([B, C * D], f32, tag="mx8")
                sm8 = smp.tile([B, C * D], f32, tag="sm8")
                nc.sync.dma_start(
                    out=mx8[:, :].rearrange("g (r f) -> g r f", f=D),
                    in_=pr3[0:B])
                nc.sync.dma_start(
                    out=sm8[:, :].rearrange("g (r f) -> g r f", f=D),
                    in_=pr3[B:2 * B])
                outt = smp.tile([B, 2 * D], f32, tag="outt")
                for buf, op, o0 in ((mx8, OP.max, 0), (sm8, OP.add, D)):
                    nc.vector.tensor_tensor(
                        out=buf[:, 0:4 * D], in0=buf[:, 0:4 * D],
                        in1=buf[:, 4 * D:8 * D], op=op)
                    nc.vector.tensor_tensor(
                        out=buf[:, 0:2 * D], in0=buf[:, 0:2 * D],
                        in1=buf[:, 2 * D:4 * D], op=op)
                    nc.vector.tensor_tensor(
                        out=outt[:, o0:o0 + D], in0=buf[:, 0:D],
                        in1=buf[:, D:2 * D], op=op)
                nc.sync.dma_start(out=out_x[:, :], in_=outt[:, :])
                if dump_T:
                    nc.sync.dma_start(out=tdump_x[:, :], in_=T[:, :])
    nc.compile()
    return nc


def _in_maps(meta, arrays):
    maps = []
    for c in range(C):
        maps.append(dict(
            tab=arrays["tab"],
            wext=arrays["wext"],
            iota=arrays["iota512"],
            iota16=arrays["iota16"],
            idxA=np.ascontiguousarray(arrays["idxA"][c]),
            cdst=np.ascontiguousarray(arrays["cdst"][c]),
            prt=np.ascontiguousarray(arrays["prt"][c]),
            cnts=np.ascontiguousarray(arrays["cnts"][c]),
            rgid=np.ascontiguousarray(arrays["rgid"][c]),
        ))
    return maps


_LAST_RESULTS = None  # stash for test harness (exec time, trace)


def kernel(**inputs):
    global _LAST_RESULTS
    import os
    meta, arrays = _preprocess(**inputs)
    nc = _build(meta)
    from concourse.bass_utils import run_bass_kernel_spmd
    res = run_bass_kernel_spmd(nc, _in_maps(meta, arrays),
                               core_ids=list(range(C)),
                               trace=bool(os.environ.get("KERNEL_TRACE")))
    _LAST_RESULTS = res
    return np.asarray(res.results[0]["out"])
